# revision 1
# baseline (speedup 1.0000x reference)
"""Trainium2 Bass kernel for nn_GAT_78151224918248 (gnn_message_passing).

Only the g0 branch of the reference is live (the g1 branch's output `ef` is
discarded), so the kernel computes
    nf0  = sqrt(2/64)*cos(feat0 @ W_rbf0 + b_rbf0)
    h0   = relu(gat_conv(nf0, g2c1_*))        # H=2, F=8
    out2 = gat_conv(h0, g2c2_*)               # H=1, F=64
    y    = MLP(relu(mean(out2, axis=0)))
and the final scalar is assembled on the host from per-core [16] partial sums
(g2c2_W is pulled out of the segment sums by linearity, so only 16-wide node
messages are aggregated on-device).

Distribution: nodes are sharded 25000/core across 8 NeuronCores (dst-major
edge sharding). Per layer, a 20-float node-record table is AllGathered; edges
are processed in 7 "rounds" by source-row range (to fit dma_gather's int16
indices), each round with a degree-sorted slot grid so all per-edge math is
plain broadcast/reduce vector work; per-round node partial sums are folded
into a DRAM accumulator with dma_scatter_add.
"""
import sys

for _p in ("/opt/trn_rl_repo", "/opt/pypackages"):
    if _p not in sys.path:
        sys.path.insert(0, _p)

import math
import numpy as np

import concourse.bass as bass
import concourse.bacc as bacc
import concourse.tile as tile
from concourse import mybir
from concourse import bass_utils

F32 = mybir.dt.float32
I16 = mybir.dt.int16
AF = mybir.ActivationFunctionType
ALU = mybir.AluOpType
AX = mybir.AxisListType

NCORES = 8
P = 128
TWO_PI = 2.0 * math.pi
PHASE_SHIFT = math.pi / 2.0
MAGIC = 12582912.0  # 1.5*2^23: fp32 add/sub rounds to nearest int

ROW = 20        # floats per table-row record
RSTRIDE = 64    # 256B stride of gatherable tables
ACC_W = 18      # floats scatter-added per node


class Cfg:
    def __init__(self, shard_real, jcount, nrounds, batch_cols=96, call_idx_cap=896):
        self.shard_real = shard_real
        self.jcount = jcount
        self.nloc = P * jcount
        assert shard_real <= self.nloc and shard_real >= (jcount - 1) * P
        self.nfull = NCORES * self.nloc
        self.nrounds = nrounds
        assert self.nfull % nrounds == 0
        self.rng_rows = self.nfull // nrounds
        assert self.rng_rows <= 32767
        self.batch_cols = batch_cols
        self.call_idx_cap = call_idx_cap
        assert call_idx_cap % P == 0
        self.call_cols = call_idx_cap // P


FULL = Cfg(shard_real=25000, jcount=196, nrounds=7)


def _wrap_idx16(vals):
    """[n] ints -> [128, ceil(n/16)] int16 SWDGE idx layout (idx i at
    [i%16, i//16], replicated to the 8 16-partition groups)."""
    n = len(vals)
    w = (n + 15) // 16
    pad = np.zeros(w * 16, dtype=np.int64)
    pad[:n] = vals
    a = np.zeros((P, w), dtype=np.int16)
    blk = pad.astype(np.int16).reshape(w, 16).T
    for g in range(8):
        a[g * 16:(g + 1) * 16, :] = blk
    return a


class Plan:
    """Host-side graph preprocessing shared by both layers."""

    def __init__(self, cfg: Cfg, src: np.ndarray, dst: np.ndarray):
        c = cfg
        self.cfg = c
        n_nodes = NCORES * c.shard_real
        src = src.astype(np.int64)
        dst = dst.astype(np.int64)
        assert src.min() >= 0 and src.max() < n_nodes
        assert dst.min() >= 0 and dst.max() < n_nodes

        core_of = dst // c.shard_real
        rows_of = (src // c.shard_real) * c.nloc + (src % c.shard_real)
        dloc = dst % c.shard_real
        rnd_of = rows_of // c.rng_rows

        # a pad table row inside every round's range (gather dummy target)
        pad_rows = np.concatenate(
            [cc * c.nloc + np.arange(c.shard_real, c.nloc) for cc in range(NCORES)])
        self.dummy = np.zeros(c.nrounds, dtype=np.int64)
        for r in range(c.nrounds):
            in_r = pad_rows[(pad_rows >= r * c.rng_rows)
                            & (pad_rows < (r + 1) * c.rng_rows)]
            assert len(in_r) > 0, f"no pad row available for round {r}"
            self.dummy[r] = in_r[0]

        # per (core, round) degree tables and node orders
        deg = np.zeros((NCORES, c.nrounds, c.nloc), dtype=np.int64)
        np.add.at(deg, (core_of, rnd_of, dloc), 1)
        orders = [[None] * c.nrounds for _ in range(NCORES)]
        for cc in range(NCORES):
            for r in range(c.nrounds):
                d = deg[cc, r]
                act = np.nonzero(d)[0]
                orders[cc][r] = act[np.argsort(-d[act], kind="stable")]

        # group templates shared across cores
        self.ng = np.zeros(c.nrounds, dtype=np.int64)
        self.widths = []
        for r in range(c.nrounds):
            ng_r = max((len(orders[cc][r]) + P - 1) // P for cc in range(NCORES))
            w_r = np.zeros(max(ng_r, 1), dtype=np.int64)[:ng_r]
            for cc in range(NCORES):
                o = orders[cc][r]
                if len(o) == 0:
                    continue
                ds = deg[cc, r][o]
                padded = np.zeros(ng_r * P, dtype=np.int64)
                padded[:len(ds)] = ds
                w_r = np.maximum(w_r, padded.reshape(ng_r, P).max(axis=1))
            self.ng[r] = ng_r
            self.widths.append(w_r)

        # batches: (round, g0, ngb, w, col0) with equal width, <= batch_cols
        self.batches = []
        self.cols = np.zeros(c.nrounds, dtype=np.int64)
        for r in range(c.nrounds):
            w_r = self.widths[r]
            col = 0
            g = 0
            while g < len(w_r):
                w = int(w_r[g])
                g2, ccols = g, 0
                while g2 < len(w_r) and int(w_r[g2]) == w and ccols + w <= c.batch_cols:
                    ccols += w
                    g2 += 1
                if g2 == g:
                    g2, ccols = g + 1, w
                self.batches.append((r, g, g2 - g, w, col))
                col += ccols
                g = g2
            self.cols[r] = col

        # per-core index arrays
        trash = c.shard_real  # local pad row for scatter padding
        self.gidx_cat, self.eridx_cat, self.scidx_cat = [], [], []
        for cc in range(NCORES):
            g_parts, er_parts, sc_parts = [], [], []
            for r in range(c.nrounds):
                ng_r = int(self.ng[r])
                w_r = self.widths[r]
                cols_r = int(self.cols[r])
                o = orders[cc][r]
                nact = len(o)
                gvals = np.full(cols_r * P, self.dummy[r], dtype=np.int64)
                ervals = np.zeros(ng_r * P, dtype=np.int64)
                scvals = np.full(ng_r * P, trash, dtype=np.int64)
                if nact:
                    ervals[:nact] = o
                    scvals[:nact] = o
                    # edges of (cc, r) sorted by node position
                    m = (core_of == cc) & (rnd_of == r)
                    ed, er_rows = dloc[m], rows_of[m]
                    pos_of = np.full(c.nloc, -1, dtype=np.int64)
                    pos_of[o] = np.arange(nact)
                    pe = pos_of[ed]
                    si = np.argsort(pe, kind="stable")
                    pe, er_rows = pe[si], er_rows[si]
                    # k = intra-node running index
                    firsts = np.searchsorted(pe, np.arange(nact))
                    k = np.arange(len(pe)) - firsts[pe]
                    col0_of_g = np.concatenate([[0], np.cumsum(w_r)])[:-1]
                    gg, pp = pe // P, pe % P
                    slot = (col0_of_g[gg] + k) * P + pp
                    gvals[slot] = er_rows
                g_parts.append(_wrap_idx16(gvals - r * c.rng_rows))
                er_parts.append(_wrap_idx16(ervals))
                sc_parts.append(_wrap_idx16(scvals))
            self.gidx_cat.append(np.concatenate(g_parts, axis=1))
            self.eridx_cat.append(np.concatenate(er_parts, axis=1))
            self.scidx_cat.append(np.concatenate(sc_parts, axis=1))
        self.gidx_off = np.concatenate(
            [[0], np.cumsum([_wrap_idx16(np.zeros(int(self.cols[r]) * P)).shape[1]
                             for r in range(c.nrounds)])])
        self.eridx_off = np.concatenate(
            [[0], np.cumsum([int(self.ng[r]) * 8 for r in range(c.nrounds)])])


def patch_dma_gather():
    import inspect
    import textwrap
    b = bass
    if getattr(b.BassGpSimd.dma_gather, "_flex_patched", False):
        return
    src = textwrap.dedent(inspect.getsource(b.BassGpSimd.dma_gather))
    bad = ("assert (\n        elem_size_bytes > 0 and elem_size_bytes % 256 == 0\n"
           "    )  # transpose restriction")
    assert bad in src, "dma_gather source changed; fix patch"
    src = src.replace(bad, "assert elem_size_bytes > 0")
    ns = dict(vars(b))
    exec(src, ns)
    ns["dma_gather"]._flex_patched = True
    b.BassGpSimd.dma_gather = ns["dma_gather"]


def _apx(base_ap, extra_off, dims):
    """New AP on the same tensor: keep partition dim, replace free dims."""
    return bass.AP(tensor=base_ap.tensor, offset=base_ap.offset + extra_off,
                   ap=[list(base_ap.ap[0])] + [list(d) for d in dims])


def build_program(cfg: Cfg, plan: Plan):
    patch_dma_gather()
    c = cfg
    J, NL, NF = c.jcount, c.nloc, c.nfull
    nc = bacc.Bacc("TRN2", target_bir_lowering=False, debug=False,
                   num_devices=NCORES)

    featT = nc.dram_tensor("featT", [64, NL], F32, kind="ExternalInput")
    waug = nc.dram_tensor("waug", [64, 64], F32, kind="ExternalInput")
    l20 = nc.dram_tensor("l20", [64, ROW], F32, kind="ExternalInput")
    b1ext = nc.dram_tensor("b1ext", [P, 16], F32, kind="ExternalInput")
    vlext = nc.dram_tensor("vlext", [P, 16], F32, kind="ExternalInput")
    vrext = nc.dram_tensor("vrext", [P, 16], F32, kind="ExternalInput")
    gidx_h = nc.dram_tensor("gidx", [P, int(plan.gidx_off[-1])], I16,
                            kind="ExternalInput")
    eridx_h = nc.dram_tensor("eridx", [P, int(plan.eridx_off[-1])], I16,
                             kind="ExternalInput")
    scidx_h = nc.dram_tensor("scidx", [P, int(plan.eridx_off[-1])], I16,
                             kind="ExternalInput")
    spartial = nc.dram_tensor("spartial", [16, 1], F32, kind="ExternalOutput")

    tab1_loc = nc.dram_tensor("tab1_loc", [NL, ROW], F32)
    tab2_loc = nc.dram_tensor("tab2_loc", [NL, ROW], F32)
    tab1_full = nc.dram_tensor("tab1_full", [NF, ROW], F32, addr_space="Shared")
    tab2_full = nc.dram_tensor("tab2_full", [NF, ROW], F32, addr_space="Shared")
    gtab1 = nc.dram_tensor("gtab1", [NF, RSTRIDE], F32)
    gtab2 = nc.dram_tensor("gtab2", [NF, RSTRIDE], F32)
    er1_loc = nc.dram_tensor("er1_loc", [NL, RSTRIDE], F32)
    er2_loc = nc.dram_tensor("er2_loc", [NL, RSTRIDE], F32)
    acc1 = nc.dram_tensor("acc1", [NL, RSTRIDE], F32)
    acc2 = nc.dram_tensor("acc2", [NL, RSTRIDE], F32)

    swdge_chain = []

    def chain(inst):
        if len(swdge_chain) >= 2:
            bass._add_dep_helper(inst.ins, swdge_chain[-2].ins, sync=True,
                                 reason="swdge ring throttle")
        swdge_chain.append(inst)
        return inst

    pad_p0 = c.shard_real - (J - 1) * P  # pads are (p >= pad_p0, j == J-1)

    with tile.TileContext(nc) as tc:
        with tc.tile_pool(name="persist", bufs=1) as pers:
            waug_sb = pers.tile([64, 64], F32)
            nc.sync.dma_start(out=waug_sb[:, :], in_=waug.ap())
            l20_sb = pers.tile([P, ROW], F32)
            nc.sync.dma_start(out=l20_sb[0:64, :], in_=l20.ap())
            nc.sync.dma_start(out=l20_sb[64:128, :], in_=l20.ap())
            b1_sb = pers.tile([P, 16], F32)
            nc.sync.dma_start(out=b1_sb[:, :], in_=b1ext.ap())
            vl_sb = pers.tile([P, 16], F32)
            nc.sync.dma_start(out=vl_sb[:, :], in_=vlext.ap())
            vr_sb = pers.tile([P, 16], F32)
            nc.sync.dma_start(out=vr_sb[:, :], in_=vrext.ap())
            zero_sb = pers.tile([P, 2048], F32)
            nc.vector.memset(zero_sb[:, :], 0.0)
            neg_sb = pers.tile([P, 2], F32)
            nc.vector.memset(neg_sb[:, :], -1.0e30)
            npad = NL - c.shard_real
            assert 0 < npad <= P
            hub1 = pers.tile([P, J, ROW], F32)
            hub2 = pers.tile([P, J, ROW], F32)

            # ---------- phase 0: RBF + layer-1 node records ----------
            half = NL // 2
            assert half % P == 0
            with tc.tile_pool(name="ph0", bufs=3) as ph0, \
                 tc.tile_pool(name="nf0p", bufs=2) as nf0p, \
                 tc.tile_pool(name="ph0ps", bufs=2, space="PSUM") as ph0ps, \
                 tc.tile_pool(name="zhps", bufs=4, space="PSUM") as zhps:
                blk = 2048 if half % 2048 == 0 else P
                nblk = half // blk
                jt_per_blk = blk // P
                for b in range(nblk):
                    c0 = b * blk
                    nf0_t = nf0p.tile([P, blk], F32, tag="nf0")
                    for ch0 in range(0, blk, 512):
                        cw = min(512, blk - ch0)
                        ft = ph0.tile([64, 512], F32, tag="ft")
                        nc.sync.dma_start(out=ft[:, :cw],
                                          in_=featT.ap()[:, c0 + ch0:c0 + ch0 + cw])
                        ft2 = ph0.tile([64, 512], F32, tag="ft2")
                        nc.sync.dma_start(
                            out=ft2[:, :cw],
                            in_=featT.ap()[:, half + c0 + ch0:half + c0 + ch0 + cw])
                        ps = ph0ps.tile([P, 512], F32, space="PSUM", tag="ps")
                        nc.tensor.matmul(ps[0:64, :cw], waug_sb[:, :], ft[:, :cw],
                                         start=True, stop=True)
                        nc.tensor.matmul(ps[64:128, :cw], waug_sb[:, :], ft2[:, :cw],
                                         start=True, stop=True)
                        wt = ph0.tile([P, 512], F32, tag="wt")
                        kt = ph0.tile([P, 512], F32, tag="kt")
                        # k = round(z / 2pi) via the fp32 magic constant
                        nc.vector.tensor_scalar(out=kt[:, :cw], in0=ps[:, :cw],
                                                scalar1=1.0 / TWO_PI, scalar2=MAGIC,
                                                op0=ALU.mult, op1=ALU.add)
                        nc.vector.tensor_scalar_add(out=kt[:, :cw], in0=kt[:, :cw],
                                                    scalar1=-MAGIC)
                        # w = z - k*2pi, clamped into the Sin LUT domain
                        nc.vector.scalar_tensor_tensor(
                            out=wt[:, :cw], in0=kt[:, :cw], scalar=-TWO_PI,
                            in1=ps[:, :cw], op0=ALU.mult, op1=ALU.add)
                        nc.vector.tensor_scalar(out=wt[:, :cw], in0=wt[:, :cw],
                                                scalar1=math.pi * 0.9999999,
                                                scalar2=-math.pi * 0.9999999,
                                                op0=ALU.min, op1=ALU.max)
                        nc.scalar.activation(nf0_t[:, ch0:ch0 + cw], wt[:, :cw],
                                             AF.Sin)
                    for hs in range(2):
                        zb = zhps.tile([P, jt_per_blk, ROW], F32, space="PSUM",
                                       tag="zb")
                        for jj in range(jt_per_blk):
                            nc.tensor.matmul(
                                zb[:, jj, :],
                                nf0_t[hs * 64:(hs + 1) * 64, jj * P:(jj + 1) * P],
                                l20_sb[hs * 64:(hs + 1) * 64, :],
                                start=True, stop=True)
                        jbase = (hs * half + c0) // P
                        nc.scalar.activation(hub1[:, jbase:jbase + jt_per_blk, :],
                                             zb[:, :, :], AF.Identity)
            nc.sync.dma_start(
                out=bass.AP(tensor=tab1_loc, offset=0,
                            ap=[[ROW, P], [ROW * P, J], [1, ROW]]),
                in_=hub1[:, :, :])
            # pad rows: el1 := -inf so padded gather slots contribute a=0
            nc.sync.dma_start(
                out=bass.AP(tensor=tab1_loc, offset=c.shard_real * ROW + 16,
                            ap=[[ROW, npad], [1, 2]]),
                in_=neg_sb[0:npad, 0:2])
            nc.sync.dma_start(
                out=bass.AP(tensor=er1_loc, offset=0,
                            ap=[[RSTRIDE, P], [RSTRIDE * P, J], [1, 2]]),
                in_=hub1[:, :, 18:20])

            nc.gpsimd.collective_compute(
                "AllGather", ALU.bypass, replica_groups=[list(range(NCORES))],
                ins=[tab1_loc.ap()], outs=[tab1_full.ap()])
            for sp0 in range(0, NF, 32768):
                spn = min(32768, NF - sp0)
                nc.sync.dma_start(
                    out=bass.AP(tensor=gtab1, offset=sp0 * RSTRIDE,
                                ap=[[RSTRIDE, spn], [1, ROW]]),
                    in_=tab1_full.ap()[sp0:sp0 + spn, :])

            # zero accumulators (runs alongside the collective)
            for a in (acc1, acc2):
                tot = NL * RSTRIDE
                flat = a.ap().rearrange("a b -> (a b)")
                step = P * 2048
                off = 0
                while off < tot:
                    sz = min(step, tot - off)
                    assert sz % P == 0
                    q = sz // P
                    v = bass.AP(tensor=a, offset=off, ap=[[q, P], [1, q]])
                    nc.sync.dma_start(out=v, in_=zero_sb[:, :q])
                    off += sz

            # ---------- edge stage ----------
            def edge_layer(layer):
                gtab = gtab1 if layer == 1 else gtab2
                er_loc = er1_loc if layer == 1 else er2_loc
                acc = acc1 if layer == 1 else acc2
                rlen = 18 if layer == 1 else 17
                nh = 2 if layer == 1 else 1
                er_view = bass.AP(tensor=er_loc, offset=0,
                                  ap=[[RSTRIDE, NL], [1, 2]])
                acc_view = bass.AP(tensor=acc, offset=0,
                                   ap=[[RSTRIDE, NL], [1, ACC_W]])
                with tc.tile_pool(name=f"l{layer}idx", bufs=2) as idxp, \
                     tc.tile_pool(name=f"l{layer}g", bufs=3) as gp, \
                     tc.tile_pool(name=f"l{layer}w", bufs=3) as wp, \
                     tc.tile_pool(name=f"l{layer}acc", bufs=2) as accp:
                    for r in range(c.nrounds):
                        ng_r = int(plan.ng[r])
                        cols_r = int(plan.cols[r])
                        if ng_r == 0:
                            continue
                        gt_view = bass.AP(
                            tensor=gtab, offset=r * c.rng_rows * RSTRIDE,
                            ap=[[RSTRIDE, c.rng_rows], [1, rlen]])
                        gi0 = int(plan.gidx_off[r])
                        ei0 = int(plan.eridx_off[r])
                        gw = (cols_r * P + 15) // 16
                        gidx_t = idxp.tile([P, max(gw, 8)], I16, tag="gidx")
                        nc.sync.dma_start(out=gidx_t[:, :gw],
                                          in_=gidx_h.ap()[:, gi0:gi0 + gw])
                        eridx_t = idxp.tile([P, max(ng_r * 8, 8)], I16, tag="eridx")
                        nc.sync.dma_start(out=eridx_t[:, :ng_r * 8],
                                          in_=eridx_h.ap()[:, ei0:ei0 + ng_r * 8])
                        scidx_t = idxp.tile([P, max(ng_r * 8, 8)], I16, tag="scidx")
                        nc.sync.dma_start(out=scidx_t[:, :ng_r * 8],
                                          in_=scidx_h.ap()[:, ei0:ei0 + ng_r * 8])

                        ert = gp.tile([P, int(max(plan.ng)), 2], F32, tag="ert")
                        for q0 in range(0, ng_r, c.call_cols):
                            qn = min(c.call_cols, ng_r - q0)
                            chain(nc.gpsimd.dma_gather(
                                ert[:, q0:q0 + qn, :], er_view,
                                eridx_t[:, q0 * 8:(q0 + qn) * 8],
                                qn * P, qn * P, 2, elem_step=RSTRIDE))

                        acc_t = accp.tile([P, int(max(plan.ng)), ACC_W], F32,
                                          tag="acc")
                        nc.vector.memset(acc_t[:, :ng_r, :], 0.0)

                        for (br, g0, ngb, w, col0) in plan.batches:
                            if br != r:
                                continue
                            cols_b = ngb * w
                            G = gp.tile([P, c.batch_cols, rlen], F32, tag="G")
                            for s0 in range(0, cols_b, c.call_cols):
                                sn = min(c.call_cols, cols_b - s0)
                                chain(nc.gpsimd.dma_gather(
                                    G[:, s0:s0 + sn, :], gt_view,
                                    gidx_t[:, (col0 + s0) * 8:(col0 + s0 + sn) * 8],
                                    sn * P, sn * P, rlen, elem_step=RSTRIDE))
                            Gb = G[:, 0:cols_b, :]
                            tt = wp.tile([P, c.batch_cols * 2], F32, tag="tt")
                            at = wp.tile([P, c.batch_cols * 2], F32, tag="at")
                            ert_b = ert[:, 0:ng_r, :]
                            if layer == 1:
                                el_ap = _apx(Gb, 16, [[rlen * w, ngb], [rlen, w],
                                                      [1, 2]])
                                er_ap = _apx(ert_b, g0 * 2, [[2, ngb], [0, w],
                                                             [1, 2]])
                                t_ap = _apx(tt[:, :], 0, [[2 * w, ngb], [1, w],
                                                          [w, 2]])
                                nact = cols_b * 2
                            else:
                                el_ap = _apx(Gb, 16, [[rlen, cols_b]])
                                er_ap = _apx(ert_b, g0 * 2, [[2, ngb], [0, w]])
                                t_ap = tt[:, 0:cols_b]
                                nact = cols_b
                            nc.vector.tensor_tensor(out=t_ap, in0=el_ap,
                                                    in1=er_ap, op=ALU.add)
                            # leaky_relu(t, 0.2) = max(0.2*t, t)
                            nc.vector.scalar_tensor_tensor(
                                out=tt[:, 0:nact], in0=tt[:, 0:nact], scalar=0.2,
                                in1=tt[:, 0:nact], op0=ALU.mult, op1=ALU.max)
                            nc.scalar.activation(at[:, 0:nact], tt[:, 0:nact],
                                                 AF.Exp)
                            V2 = wp.tile([P, c.batch_cols, 16], F32, tag="V2")
                            for hd in range(nh):
                                fw = 16 // nh
                                h_ap = _apx(Gb, hd * fw, [[rlen * w, ngb],
                                                          [rlen, w], [1, fw]])
                                if layer == 1:
                                    a_ap = _apx(at[:, :], hd * w,
                                                [[2 * w, ngb], [1, w], [0, fw]])
                                    v_ap = _apx(V2[:, :, :], hd * fw * w,
                                                [[16 * w, ngb], [1, w], [w, fw]])
                                else:
                                    a_ap = _apx(at[:, :], 0,
                                                [[w, ngb], [1, w], [0, fw]])
                                    v_ap = _apx(V2[:, :, :], 0,
                                                [[16 * w, ngb], [1, w], [w, fw]])
                                nc.vector.tensor_tensor(out=v_ap, in0=h_ap,
                                                        in1=a_ap, op=ALU.mult)
                            vred = _apx(V2[:, :, :], 0, [[16 * w, ngb], [w, 16],
                                                         [1, w]])
                            m_ap = _apx(acc_t[:, :, :], g0 * ACC_W,
                                        [[ACC_W, ngb], [1, 16]])
                            nc.vector.tensor_reduce(out=m_ap, in_=vred, axis=AX.X,
                                                    op=ALU.add)
                            if layer == 1:
                                den_in = _apx(at[:, :], 0, [[2 * w, ngb], [w, 2],
                                                            [1, w]])
                                den_out = _apx(acc_t[:, :, :], g0 * ACC_W + 16,
                                               [[ACC_W, ngb], [1, 2]])
                            else:
                                den_in = _apx(at[:, :], 0, [[w, ngb], [1, w]])
                                den_out = _apx(acc_t[:, :, :], g0 * ACC_W + 16,
                                               [[ACC_W, ngb]])
                            nc.vector.tensor_reduce(out=den_out, in_=den_in,
                                                    axis=AX.X, op=ALU.add)

                        for q0 in range(0, ng_r, c.call_cols):
                            qn = min(c.call_cols, ng_r - q0)
                            chain(nc.gpsimd.dma_scatter_add(
                                acc_view, acc_t[:, q0:q0 + qn, :],
                                scidx_t[:, q0 * 8:(q0 + qn) * 8],
                                qn * P, qn * P, ACC_W, elem_step=RSTRIDE))

            edge_layer(1)

            # ---------- layer-1 finalize: h0 / el2 / er2 -> table2 ----------
            accl = pers.tile([P, J, ACC_W], F32)
            nc.sync.dma_start(
                out=bass.AP(tensor=acc1, offset=c.shard_real * RSTRIDE,
                            ap=[[RSTRIDE, npad], [1, RSTRIDE]]),
                in_=zero_sb[0:npad, 0:RSTRIDE])
            nc.sync.dma_start(
                out=accl[:, :, :],
                in_=bass.AP(tensor=acc1, offset=0,
                            ap=[[RSTRIDE, P], [RSTRIDE * P, J], [1, ACC_W]]))
            dmax = pers.tile([P, J, 2], F32)
            nc.vector.tensor_scalar_max(out=dmax[:, :, :], in0=accl[:, :, 16:18],
                                        scalar1=1e-9)
            rec = pers.tile([P, J, 2], F32)
            nc.vector.reciprocal(out=rec[:, :, :], in_=dmax[:, :, :])
            h0p = pers.tile([P, J, 16], F32)
            rec_b = _apx(rec[:, :, :], 0, [[2, J], [1, 2], [0, 8]])
            nc.vector.tensor_tensor(out=h0p[:, :, :], in0=accl[:, :, 0:16],
                                    in1=rec_b, op=ALU.mult)
            b1_b = _apx(b1_sb[:, :], 0, [[0, J], [1, 16]])
            nc.vector.tensor_tensor(out=h0p[:, :, :], in0=h0p[:, :, :], in1=b1_b,
                                    op=ALU.add)
            nc.vector.tensor_scalar_max(out=hub2[:, :, 0:16], in0=h0p[:, :, :],
                                        scalar1=0.0)
            tmp = pers.tile([P, J, 16], F32)
            vl_b = _apx(vl_sb[:, :], 0, [[0, J], [1, 16]])
            nc.vector.tensor_tensor(out=tmp[:, :, :], in0=hub2[:, :, 0:16],
                                    in1=vl_b, op=ALU.mult)
            nc.vector.tensor_reduce(out=hub2[:, :, 16], in_=tmp[:, :, :],
                                    axis=AX.X, op=ALU.add)
            vr_b = _apx(vr_sb[:, :], 0, [[0, J], [1, 16]])
            nc.vector.tensor_tensor(out=tmp[:, :, :], in0=hub2[:, :, 0:16],
                                    in1=vr_b, op=ALU.mult)
            nc.vector.tensor_reduce(out=hub2[:, :, 17], in_=tmp[:, :, :],
                                    axis=AX.X, op=ALU.add)
            nc.vector.memset(hub2[:, :, 18:20], 0.0)
            nc.sync.dma_start(
                out=bass.AP(tensor=tab2_loc, offset=0,
                            ap=[[ROW, P], [ROW * P, J], [1, ROW]]),
                in_=hub2[:, :, :])
            nc.sync.dma_start(
                out=bass.AP(tensor=tab2_loc, offset=c.shard_real * ROW + 16,
                            ap=[[ROW, npad], [1, 1]]),
                in_=neg_sb[0:npad, 0:1])
            nc.sync.dma_start(
                out=bass.AP(tensor=er2_loc, offset=0,
                            ap=[[RSTRIDE, P], [RSTRIDE * P, J], [1, 2]]),
                in_=hub2[:, :, 17:19])

            nc.gpsimd.collective_compute(
                "AllGather", ALU.bypass, replica_groups=[list(range(NCORES))],
                ins=[tab2_loc.ap()], outs=[tab2_full.ap()])
            for sp0 in range(0, NF, 32768):
                spn = min(32768, NF - sp0)
                nc.sync.dma_start(
                    out=bass.AP(tensor=gtab2, offset=sp0 * RSTRIDE,
                                ap=[[RSTRIDE, spn], [1, ROW]]),
                    in_=tab2_full.ap()[sp0:sp0 + spn, :])

            edge_layer(2)

            # ---------- layer-2 finalize -> spartial ----------
            acc2l = pers.tile([P, J, 17], F32)
            nc.sync.dma_start(
                out=bass.AP(tensor=acc2, offset=c.shard_real * RSTRIDE,
                            ap=[[RSTRIDE, npad], [1, RSTRIDE]]),
                in_=zero_sb[0:npad, 0:RSTRIDE])
            nc.sync.dma_start(
                out=acc2l[:, :, :],
                in_=bass.AP(tensor=acc2, offset=0,
                            ap=[[RSTRIDE, P], [RSTRIDE * P, J], [1, 17]]))
            d2 = pers.tile([P, J], F32)
            nc.vector.tensor_scalar_max(out=d2[:, :], in0=acc2l[:, :, 16],
                                        scalar1=1e-9)
            r2 = pers.tile([P, J], F32)
            nc.vector.reciprocal(out=r2[:, :], in_=d2[:, :])
            rt = pers.tile([P, 16, J], F32)
            r2_b = _apx(r2[:, :], 0, [[1, J], [0, 16]])
            rt_ap = _apx(rt[:, :, :], 0, [[1, J], [J, 16]])
            nc.vector.tensor_tensor(out=rt_ap, in0=acc2l[:, :, 0:16], in1=r2_b,
                                    op=ALU.mult)
            S_acc = pers.tile([P, 16], F32)
            nc.vector.tensor_reduce(out=S_acc[:, :], in_=rt[:, :, :], axis=AX.X,
                                    op=ALU.add)
            ones = pers.tile([P, 1], F32)
            nc.vector.memset(ones[:, :], 1.0)
            with tc.tile_pool(name="fps", bufs=1, space="PSUM") as fps:
                sp = fps.tile([16, 1], F32, space="PSUM")
                nc.tensor.matmul(sp[:, :], S_acc[:, :], ones[:, :], start=True,
                                 stop=True)
                sout = pers.tile([16, 1], F32)
                nc.vector.tensor_copy(out=sout[:, :], in_=sp[:, :])
                nc.sync.dma_start(out=spartial.ap(), in_=sout[:, :])

    nc.compile()
    return nc


# ---------------- host orchestration ----------------

_CACHE = {}


def _get(cfg, src0, dst0):
    key = (cfg.shard_real, cfg.jcount, cfg.nrounds,
           hash(src0.tobytes()), hash(dst0.tobytes()))
    if key not in _CACHE:
        plan = Plan(cfg, src0, dst0)
        nc = build_program(cfg, plan)
        _CACHE[key] = (plan, nc)
    return _CACHE[key]


def make_in_maps(cfg, plan, inputs):
    c = cfg
    s = math.sqrt(2.0 / 64.0)
    feat0 = np.asarray(inputs["feat0"], dtype=np.float32)
    W_rbf0 = np.asarray(inputs["W_rbf0"], dtype=np.float32)
    b_rbf0 = np.asarray(inputs["b_rbf0"], dtype=np.float32)
    g2c1_W = np.asarray(inputs["g2c1_W"], dtype=np.float32)
    g2c1_al = np.asarray(inputs["g2c1_al"], dtype=np.float32)
    g2c1_ar = np.asarray(inputs["g2c1_ar"], dtype=np.float32)
    g2c1_b = np.asarray(inputs["g2c1_b"], dtype=np.float32)
    g2c2_W = np.asarray(inputs["g2c2_W"], dtype=np.float32)
    g2c2_al = np.asarray(inputs["g2c2_al"], dtype=np.float32)
    g2c2_ar = np.asarray(inputs["g2c2_ar"], dtype=np.float32)

    dfeat = feat0.shape[1]
    waug = np.zeros((64, 64), dtype=np.float32)
    waug[:dfeat, :] = W_rbf0
    waug[dfeat, :] = b_rbf0 + PHASE_SHIFT
    al16 = np.zeros((16, 2), dtype=np.float32)
    ar16 = np.zeros((16, 2), dtype=np.float32)
    for hd in range(2):
        al16[hd * 8:(hd + 1) * 8, hd] = g2c1_al[hd]
        ar16[hd * 8:(hd + 1) * 8, hd] = g2c1_ar[hd]
    l20 = np.zeros((64, ROW), dtype=np.float32)
    l20[:, 0:16] = s * g2c1_W
    l20[:, 16:18] = s * (g2c1_W @ al16)
    l20[:, 18:20] = s * (g2c1_W @ ar16)
    vl = (g2c2_W @ g2c2_al[0]).astype(np.float32)
    vr = (g2c2_W @ g2c2_ar[0]).astype(np.float32)

    maps = []
    for cc in range(NCORES):
        ft = np.zeros((64, c.nloc), dtype=np.float32)
        lo = cc * c.shard_real
        ft[:dfeat, :c.shard_real] = feat0[lo:lo + c.shard_real].T
        ft[dfeat, :] = 1.0
        maps.append({
            "featT": ft,
            "waug": waug,
            "l20": l20,
            "b1ext": np.tile(g2c1_b.reshape(1, 16), (P, 1)),
            "vlext": np.tile(vl.reshape(1, 16), (P, 1)),
            "vrext": np.tile(vr.reshape(1, 16), (P, 1)),
            "gidx": plan.gidx_cat[cc],
            "eridx": plan.eridx_cat[cc],
            "scidx": plan.scidx_cat[cc],
        })
    return maps


def host_tail(cfg, inputs, spartials):
    S = np.zeros(16, dtype=np.float64)
    for cc in range(NCORES):
        S += spartials[cc][:, 0].astype(np.float64)
    n_nodes = NCORES * cfg.shard_real
    W2 = np.asarray(inputs["g2c2_W"], dtype=np.float64)
    b2 = np.asarray(inputs["g2c2_b"], dtype=np.float64)
    mean = (S @ W2) / n_nodes + b2
    h = np.maximum(mean, 0.0)
    h = np.maximum(
        h @ np.asarray(inputs["fc1_w"], dtype=np.float64).T
        + np.asarray(inputs["fc1_b"], dtype=np.float64), 0.0)
    out = (h @ np.asarray(inputs["out_w"], dtype=np.float64).T
           + np.asarray(inputs["out_b"], dtype=np.float64))
    return out.astype(np.float32).reshape(1)


def kernel(**inputs):
    cfg = FULL
    src0 = np.asarray(inputs["src0"])
    dst0 = np.asarray(inputs["dst0"])
    plan, nc = _get(cfg, src0, dst0)
    in_maps = make_in_maps(cfg, plan, inputs)
    res = bass_utils.run_bass_kernel_spmd(nc, in_maps,
                                          core_ids=list(range(NCORES)))
    return host_tail(cfg, inputs, [res.results[cc]["spartial"]
                                   for cc in range(NCORES)])



# revision 10
# speedup vs baseline: 1.0919x; 1.0919x over previous
"""Trainium2 Bass kernel for nn_GAT_78151224918248 (gnn_message_passing).

Only the g0 branch of the reference is live (the g1 branch's output `ef` is
discarded), so the kernel computes
    nf0  = sqrt(2/64)*cos(feat0 @ W_rbf0 + b_rbf0)
    h0   = relu(gat_conv(nf0, g2c1_*))        # H=2, F=8
    out2 = gat_conv(h0, g2c2_*)               # H=1, F=64
    y    = MLP(relu(mean(out2, axis=0)))
and the final scalar is assembled on the host from per-core [16] partial sums
(g2c2_W is pulled out of the segment sums by linearity, so only 16-wide node
messages are aggregated on-device).

Distribution: nodes are sharded 25000/core across 8 NeuronCores (dst-major
edge sharding). Per layer, a 20-float node-record table is AllGathered; edges
are processed in 7 "rounds" by source-row range (to fit dma_gather's int16
indices), each round with a degree-sorted slot grid so all per-edge math is
plain broadcast/reduce vector work; per-round node partial sums are folded
into a DRAM accumulator with dma_scatter_add. Gather/scatter calls are one
per round (SWDGE streams descriptors through the ring with backpressure).
"""
import sys

for _p in ("/opt/trn_rl_repo", "/opt/pypackages"):
    if _p not in sys.path:
        sys.path.insert(0, _p)

import math
import numpy as np

import concourse.bass as bass
import concourse.bacc as bacc
import concourse.tile as tile
from concourse import mybir
from concourse import bass_utils

F32 = mybir.dt.float32
I16 = mybir.dt.int16
AF = mybir.ActivationFunctionType
ALU = mybir.AluOpType
AX = mybir.AxisListType

NCORES = 8
P = 128
TWO_PI = 2.0 * math.pi
PHASE_SHIFT = math.pi / 2.0
MAGIC = 12582912.0  # 1.5*2^23: fp32 add/sub rounds to nearest int

ROW = 20        # floats per table-row record
RSTRIDE = 64    # 256B stride of gatherable tables
ACC_W = 18      # floats scatter-added per node


class Cfg:
    def __init__(self, shard_real, jcount, nrounds):
        self.shard_real = shard_real
        self.jcount = jcount
        self.nloc = P * jcount
        assert shard_real <= self.nloc and shard_real >= (jcount - 1) * P
        self.nfull = NCORES * self.nloc
        self.nrounds = nrounds
        assert self.nfull % nrounds == 0
        self.rng_rows = self.nfull // nrounds
        assert self.rng_rows <= 32767


FULL = Cfg(shard_real=25000, jcount=196, nrounds=7)


def _wrap_idx16(vals):
    """[n] ints -> [128, ceil(n/16)] int16 SWDGE idx layout (idx i at
    [i%16, i//16], replicated to the 8 16-partition groups)."""
    n = len(vals)
    w = (n + 15) // 16
    pad = np.zeros(w * 16, dtype=np.int64)
    pad[:n] = vals
    a = np.zeros((P, w), dtype=np.int16)
    blk = pad.astype(np.int16).reshape(w, 16).T
    for g in range(8):
        a[g * 16:(g + 1) * 16, :] = blk
    return a


class Plan:
    """Host-side graph preprocessing shared by both layers."""

    def __init__(self, cfg: Cfg, src: np.ndarray, dst: np.ndarray):
        c = cfg
        self.cfg = c
        n_nodes = NCORES * c.shard_real
        src = src.astype(np.int64)
        dst = dst.astype(np.int64)
        assert src.min() >= 0 and src.max() < n_nodes
        assert dst.min() >= 0 and dst.max() < n_nodes

        core_of = dst // c.shard_real
        rows_of = (src // c.shard_real) * c.nloc + (src % c.shard_real)
        dloc = dst % c.shard_real
        rnd_of = rows_of // c.rng_rows

        # a pad table row inside every round's range (gather dummy target)
        pad_rows = np.concatenate(
            [cc * c.nloc + np.arange(c.shard_real, c.nloc) for cc in range(NCORES)])
        self.dummy = np.zeros(c.nrounds, dtype=np.int64)
        for r in range(c.nrounds):
            in_r = pad_rows[(pad_rows >= r * c.rng_rows)
                            & (pad_rows < (r + 1) * c.rng_rows)]
            assert len(in_r) > 0, f"no pad row available for round {r}"
            self.dummy[r] = in_r[0]

        # per (core, round) degree tables and node orders
        deg = np.zeros((NCORES, c.nrounds, c.nloc), dtype=np.int64)
        np.add.at(deg, (core_of, rnd_of, dloc), 1)
        orders = [[None] * c.nrounds for _ in range(NCORES)]
        for cc in range(NCORES):
            for r in range(c.nrounds):
                d = deg[cc, r]
                act = np.nonzero(d)[0]
                orders[cc][r] = act[np.argsort(-d[act], kind="stable")]

        # group templates shared across cores
        self.ng = np.zeros(c.nrounds, dtype=np.int64)
        self.widths = []
        for r in range(c.nrounds):
            ng_r = max((len(orders[cc][r]) + P - 1) // P for cc in range(NCORES))
            w_r = np.zeros(max(ng_r, 1), dtype=np.int64)[:ng_r]
            for cc in range(NCORES):
                o = orders[cc][r]
                if len(o) == 0:
                    continue
                ds = deg[cc, r][o]
                padded = np.zeros(ng_r * P, dtype=np.int64)
                padded[:len(ds)] = ds
                w_r = np.maximum(w_r, padded.reshape(ng_r, P).max(axis=1))
            self.ng[r] = ng_r
            self.widths.append(w_r)

        # batches: (round, g0, ngb, w, col0) = maximal equal-width group runs
        self.batches = []
        self.cols = np.zeros(c.nrounds, dtype=np.int64)
        for r in range(c.nrounds):
            w_r = self.widths[r]
            col = 0
            g = 0
            while g < len(w_r):
                w = int(w_r[g])
                g2 = g
                while g2 < len(w_r) and int(w_r[g2]) == w:
                    g2 += 1
                self.batches.append((r, g, g2 - g, w, col))
                col += (g2 - g) * w
                g = g2
            self.cols[r] = col

        # per-core index arrays
        trash = c.shard_real  # local pad row for scatter padding
        self.gidx_cat, self.eridx_cat, self.scidx_cat = [], [], []
        for cc in range(NCORES):
            g_parts, er_parts, sc_parts = [], [], []
            for r in range(c.nrounds):
                ng_r = int(self.ng[r])
                w_r = self.widths[r]
                cols_r = int(self.cols[r])
                o = orders[cc][r]
                nact = len(o)
                gvals = np.full(cols_r * P, self.dummy[r], dtype=np.int64)
                ervals = np.zeros(ng_r * P, dtype=np.int64)
                scvals = np.full(ng_r * P, trash, dtype=np.int64)
                if nact:
                    ervals[:nact] = o
                    scvals[:nact] = o
                    # edges of (cc, r) sorted by node position
                    m = (core_of == cc) & (rnd_of == r)
                    ed, er_rows = dloc[m], rows_of[m]
                    pos_of = np.full(c.nloc, -1, dtype=np.int64)
                    pos_of[o] = np.arange(nact)
                    pe = pos_of[ed]
                    si = np.argsort(pe, kind="stable")
                    pe, er_rows = pe[si], er_rows[si]
                    # k = intra-node running index
                    firsts = np.searchsorted(pe, np.arange(nact))
                    k = np.arange(len(pe)) - firsts[pe]
                    col0_of_g = np.concatenate([[0], np.cumsum(w_r)])[:-1]
                    gg, pp = pe // P, pe % P
                    slot = (col0_of_g[gg] + k) * P + pp
                    gvals[slot] = er_rows
                g_parts.append(_wrap_idx16(gvals - r * c.rng_rows))
                er_parts.append(_wrap_idx16(ervals))
                sc_parts.append(_wrap_idx16(scvals))
            self.gidx_cat.append(np.concatenate(g_parts, axis=1))
            self.eridx_cat.append(np.concatenate(er_parts, axis=1))
            self.scidx_cat.append(np.concatenate(sc_parts, axis=1))
        self.gidx_off = np.concatenate(
            [[0], np.cumsum([_wrap_idx16(np.zeros(int(self.cols[r]) * P)).shape[1]
                             for r in range(c.nrounds)])])
        self.eridx_off = np.concatenate(
            [[0], np.cumsum([int(self.ng[r]) * 8 for r in range(c.nrounds)])])


def patch_dma_gather():
    import inspect
    import textwrap
    b = bass
    if getattr(b.BassGpSimd.dma_gather, "_flex_patched", False):
        return
    src = textwrap.dedent(inspect.getsource(b.BassGpSimd.dma_gather))
    bad = ("assert (\n        elem_size_bytes > 0 and elem_size_bytes % 256 == 0\n"
           "    )  # transpose restriction")
    assert bad in src, "dma_gather source changed; fix patch"
    src = src.replace(bad, "assert elem_size_bytes > 0")
    ns = dict(vars(b))
    exec(src, ns)
    ns["dma_gather"]._flex_patched = True
    b.BassGpSimd.dma_gather = ns["dma_gather"]


def _apx(base_ap, extra_off, dims):
    """New AP on the same tensor: keep partition dim, replace free dims."""
    return bass.AP(tensor=base_ap.tensor, offset=base_ap.offset + extra_off,
                   ap=[list(base_ap.ap[0])] + [list(d) for d in dims])


def build_program(cfg: Cfg, plan: Plan, nreps: int = 1):
    patch_dma_gather()
    c = cfg
    J, NL, NF = c.jcount, c.nloc, c.nfull
    ng_max = int(max(plan.ng))
    cols_max = int(max(plan.cols))
    nc = bacc.Bacc("TRN2", target_bir_lowering=False, debug=False,
                   num_devices=NCORES, dynamic_dma_scratch_size=65536)

    featT = nc.dram_tensor("featT", [64, NL], F32, kind="ExternalInput")
    waug = nc.dram_tensor("waug", [64, 64], F32, kind="ExternalInput")
    l20 = nc.dram_tensor("l20", [64, ROW], F32, kind="ExternalInput")
    b1ext = nc.dram_tensor("b1ext", [P, 16], F32, kind="ExternalInput")
    vlext = nc.dram_tensor("vlext", [P, 16], F32, kind="ExternalInput")
    vrext = nc.dram_tensor("vrext", [P, 16], F32, kind="ExternalInput")
    gidx_h = nc.dram_tensor("gidx", [P, int(plan.gidx_off[-1])], I16,
                            kind="ExternalInput")
    eridx_h = nc.dram_tensor("eridx", [P, int(plan.eridx_off[-1])], I16,
                             kind="ExternalInput")
    scidx_h = nc.dram_tensor("scidx", [P, int(plan.eridx_off[-1])], I16,
                             kind="ExternalInput")
    spartial = nc.dram_tensor("spartial", [16, 1], F32, kind="ExternalOutput")

    tab1_loc = nc.dram_tensor("tab1_loc", [NL, ROW], F32)
    tab2_loc = nc.dram_tensor("tab2_loc", [NL, ROW], F32)
    tab1_full = nc.dram_tensor("tab1_full", [NF, ROW], F32, addr_space="Shared")
    tab2_full = nc.dram_tensor("tab2_full", [NF, ROW], F32, addr_space="Shared")
    gtab1 = nc.dram_tensor("gtab1", [NF, RSTRIDE], F32)
    gtab2 = nc.dram_tensor("gtab2", [NF, RSTRIDE], F32)
    er1_loc = nc.dram_tensor("er1_loc", [NL, RSTRIDE], F32)
    er2_loc = nc.dram_tensor("er2_loc", [NL, RSTRIDE], F32)
    acc1 = nc.dram_tensor("acc1", [NL, RSTRIDE], F32)
    acc2 = nc.dram_tensor("acc2", [NL, RSTRIDE], F32)

    pad_p0 = c.shard_real - (J - 1) * P  # pads are (p >= pad_p0, j == J-1)
    npad = NL - c.shard_real
    assert 0 < npad <= P

    with tile.TileContext(nc) as tc:
        with tc.tile_pool(name="persist", bufs=1) as pers:
            waug_sb = pers.tile([64, 64], F32)
            nc.sync.dma_start(out=waug_sb[:, :], in_=waug.ap())
            l20_sb = pers.tile([P, ROW], F32)
            nc.sync.dma_start(out=l20_sb[0:64, :], in_=l20.ap())
            nc.sync.dma_start(out=l20_sb[64:128, :], in_=l20.ap())
            b1_sb = pers.tile([P, 16], F32)
            nc.sync.dma_start(out=b1_sb[:, :], in_=b1ext.ap())
            vl_sb = pers.tile([P, 16], F32)
            nc.sync.dma_start(out=vl_sb[:, :], in_=vlext.ap())
            vr_sb = pers.tile([P, 16], F32)
            nc.sync.dma_start(out=vr_sb[:, :], in_=vrext.ap())
            zero_sb = pers.tile([P, 2048], F32)
            nc.vector.memset(zero_sb[:, :], 0.0)
            # pad-row record for layer1/layer2 tables: zeros except el=-1e30
            pad1_sb = pers.tile([P, ROW], F32)
            nc.vector.memset(pad1_sb[:, :], 0.0)
            nc.vector.memset(pad1_sb[:, 16:18], -1.0e30)
            pad2_sb = pers.tile([P, ROW], F32)
            nc.vector.memset(pad2_sb[:, :], 0.0)
            nc.vector.memset(pad2_sb[:, 16:17], -1.0e30)
            hub1 = pers.tile([P, J, ROW], F32)
            hub2 = pers.tile([P, J, ROW], F32)

            T = dict(featT=featT, waug_sb=waug_sb, l20_sb=l20_sb,
                     b1_sb=b1_sb, vl_sb=vl_sb, vr_sb=vr_sb,
                     zero_sb=zero_sb, pad1_sb=pad1_sb,
                     pad2_sb=pad2_sb, hub1=hub1, hub2=hub2,
                     gidx_h=gidx_h, eridx_h=eridx_h, scidx_h=scidx_h,
                     tab1_loc=tab1_loc, tab2_loc=tab2_loc,
                     tab1_full=tab1_full, tab2_full=tab2_full,
                     gtab1=gtab1, gtab2=gtab2, er1_loc=er1_loc,
                     er2_loc=er2_loc, acc1=acc1, acc2=acc2,
                     spartial=spartial)
            for rep in range(nreps):
                body_once(nc, tc, c, plan, rep, T, ng_max, cols_max)

    nc.compile()
    return nc


CALL_COLS = 28  # 3584 indices per SWDGE call (ring is 4096 descriptors)


def body_once(nc, tc, c, plan, rep, T, ng_max, cols_max):
    J, NL, NF = c.jcount, c.nloc, c.nfull
    npad = NL - c.shard_real

    swdge_chain = T.setdefault("swdge_chain", [])

    def chain(inst):
        if len(swdge_chain) >= 2:
            bass._add_dep_helper(inst.ins, swdge_chain[-2].ins, sync=True,
                                 reason="swdge ring throttle")
        swdge_chain.append(inst)
        return inst
    featT, zero_sb = T["featT"], T["zero_sb"]
    hub1, hub2 = T["hub1"], T["hub2"]
    tab1_loc, tab2_loc = T["tab1_loc"], T["tab2_loc"]
    tab1_full, tab2_full = T["tab1_full"], T["tab2_full"]
    gtab1, gtab2 = T["gtab1"], T["gtab2"]
    er1_loc, er2_loc = T["er1_loc"], T["er2_loc"]
    acc1, acc2 = T["acc1"], T["acc2"]

    # ---------- phase 0: RBF + layer-1 node records ----------
    half = NL // 2
    assert half % P == 0
    blk = 1792 if half % 1792 == 0 else P
    nblk = half // blk
    jt_per_blk = blk // P
    with tc.tile_pool(name=f"ph0_{rep}", bufs=3) as ph0, \
         tc.tile_pool(name=f"nf0p_{rep}", bufs=2) as nf0p, \
         tc.tile_pool(name=f"ph0ps_{rep}", bufs=2, space="PSUM") as ph0ps, \
         tc.tile_pool(name=f"zhps_{rep}", bufs=4, space="PSUM") as zhps:
        for b in range(nblk):
            c0 = b * blk
            nf0_t = nf0p.tile([P, blk], F32, tag="nf0")
            for ch0 in range(0, blk, 512):
                cw = min(512, blk - ch0)
                ft = ph0.tile([64, 512], F32, tag="ft")
                nc.sync.dma_start(out=ft[:, :cw],
                                  in_=featT.ap()[:, c0 + ch0:c0 + ch0 + cw])
                ft2 = ph0.tile([64, 512], F32, tag="ft2")
                nc.sync.dma_start(
                    out=ft2[:, :cw],
                    in_=featT.ap()[:, half + c0 + ch0:half + c0 + ch0 + cw])
                ps = ph0ps.tile([P, 512], F32, space="PSUM", tag="ps")
                nc.tensor.matmul(ps[0:64, :cw], T["waug_sb"][:, :], ft[:, :cw],
                                 start=True, stop=True)
                nc.tensor.matmul(ps[64:128, :cw], T["waug_sb"][:, :],
                                 ft2[:, :cw], start=True, stop=True)
                wt = ph0.tile([P, 512], F32, tag="wt")
                kt = ph0.tile([P, 512], F32, tag="kt")
                # k = round(z / 2pi) via the fp32 magic constant
                nc.vector.tensor_scalar(out=kt[:, :cw], in0=ps[:, :cw],
                                        scalar1=1.0 / TWO_PI, scalar2=MAGIC,
                                        op0=ALU.mult, op1=ALU.add)
                nc.vector.tensor_scalar_add(out=kt[:, :cw], in0=kt[:, :cw],
                                            scalar1=-MAGIC)
                # w = z - k*2pi, clamped into the Sin LUT domain
                nc.vector.scalar_tensor_tensor(
                    out=wt[:, :cw], in0=kt[:, :cw], scalar=-TWO_PI,
                    in1=ps[:, :cw], op0=ALU.mult, op1=ALU.add)
                nc.vector.tensor_scalar(out=wt[:, :cw], in0=wt[:, :cw],
                                        scalar1=math.pi * 0.9999999,
                                        scalar2=-math.pi * 0.9999999,
                                        op0=ALU.min, op1=ALU.max)
                nc.scalar.activation(nf0_t[:, ch0:ch0 + cw], wt[:, :cw],
                                     AF.Sin)
            for hs in range(2):
                zb = zhps.tile([P, jt_per_blk, ROW], F32, space="PSUM",
                               tag="zb")
                for jj in range(jt_per_blk):
                    nc.tensor.matmul(
                        zb[:, jj, :],
                        nf0_t[hs * 64:(hs + 1) * 64, jj * P:(jj + 1) * P],
                        T["l20_sb"][hs * 64:(hs + 1) * 64, :],
                        start=True, stop=True)
                jbase = (hs * half + c0) // P
                nc.scalar.activation(hub1[:, jbase:jbase + jt_per_blk, :],
                                     zb[:, :, :], AF.Identity)
    nc.sync.dma_start(
        out=bass.AP(tensor=tab1_loc, offset=0,
                    ap=[[ROW, P], [ROW * P, J], [1, ROW]]),
        in_=hub1[:, :, :])
    # pad rows: zero h, el := -inf so padded gather slots contribute a=0
    nc.sync.dma_start(
        out=bass.AP(tensor=tab1_loc, offset=c.shard_real * ROW,
                    ap=[[ROW, npad], [1, ROW]]),
        in_=T["pad1_sb"][0:npad, :])
    nc.sync.dma_start(
        out=bass.AP(tensor=er1_loc, offset=0,
                    ap=[[RSTRIDE, P], [RSTRIDE * P, J], [1, 2]]),
        in_=hub1[:, :, 18:20])

    nc.gpsimd.collective_compute(
        "AllGather", ALU.bypass, replica_groups=[list(range(NCORES))],
        ins=[tab1_loc.ap()], outs=[tab1_full.ap()])
    for sp0 in range(0, NF, 32768):
        spn = min(32768, NF - sp0)
        nc.sync.dma_start(
            out=bass.AP(tensor=gtab1, offset=sp0 * RSTRIDE,
                        ap=[[RSTRIDE, spn], [1, ROW]]),
            in_=tab1_full.ap()[sp0:sp0 + spn, :])

    # zero accumulators (runs alongside the collective)
    for a in (acc1, acc2):
        tot = NL * RSTRIDE
        step = P * 2048
        off = 0
        while off < tot:
            sz = min(step, tot - off)
            assert sz % P == 0
            q = sz // P
            v = bass.AP(tensor=a, offset=off, ap=[[q, P], [1, q]])
            nc.sync.dma_start(out=v, in_=zero_sb[:, :q])
            off += sz

    # ---------- edge stage ----------
    def edge_layer(layer):
        gtab = gtab1 if layer == 1 else gtab2
        er_loc = er1_loc if layer == 1 else er2_loc
        acc = acc1 if layer == 1 else acc2
        rlen = 18 if layer == 1 else 17
        nh = 2 if layer == 1 else 1
        er_view = bass.AP(tensor=er_loc, offset=0,
                          ap=[[RSTRIDE, NL], [1, 2]])
        acc_view = bass.AP(tensor=acc, offset=0,
                           ap=[[RSTRIDE, NL], [1, ACC_W]])
        with tc.tile_pool(name=f"l{layer}idx_{rep}", bufs=2) as idxp, \
             tc.tile_pool(name=f"l{layer}g_{rep}", bufs=2) as gp, \
             tc.tile_pool(name=f"l{layer}w_{rep}", bufs=2) as wp, \
             tc.tile_pool(name=f"l{layer}acc_{rep}", bufs=2) as accp:
            for r in range(c.nrounds):
                ng_r = int(plan.ng[r])
                cols_r = int(plan.cols[r])
                if ng_r == 0:
                    continue
                gt_view = bass.AP(
                    tensor=gtab, offset=r * c.rng_rows * RSTRIDE,
                    ap=[[RSTRIDE, c.rng_rows], [1, rlen]])
                gi0 = int(plan.gidx_off[r])
                ei0 = int(plan.eridx_off[r])
                gw = (cols_r * P + 15) // 16
                gidx_t = idxp.tile([P, (cols_max * P + 15) // 16], I16,
                                   tag="gidx")
                nc.sync.dma_start(out=gidx_t[:, :gw],
                                  in_=T["gidx_h"].ap()[:, gi0:gi0 + gw])
                eridx_t = idxp.tile([P, ng_max * 8], I16, tag="eridx")
                nc.sync.dma_start(out=eridx_t[:, :ng_r * 8],
                                  in_=T["eridx_h"].ap()[:, ei0:ei0 + ng_r * 8])
                scidx_t = idxp.tile([P, ng_max * 8], I16, tag="scidx")
                nc.sync.dma_start(out=scidx_t[:, :ng_r * 8],
                                  in_=T["scidx_h"].ap()[:, ei0:ei0 + ng_r * 8])

                ert = gp.tile([P, ng_max, 2], F32, tag="ert")
                for q0 in range(0, ng_r, CALL_COLS):
                    qn = min(CALL_COLS, ng_r - q0)
                    chain(nc.gpsimd.dma_gather(
                        ert[:, q0:q0 + qn, :], er_view,
                        eridx_t[:, q0 * 8:(q0 + qn) * 8],
                        qn * P, qn * P, 2, elem_step=RSTRIDE,
                        single_packet=False))

                G = gp.tile([P, cols_max, rlen], F32, tag="G")
                for q0 in range(0, cols_r, CALL_COLS):
                    qn = min(CALL_COLS, cols_r - q0)
                    chain(nc.gpsimd.dma_gather(
                        G[:, q0:q0 + qn, :], gt_view,
                        gidx_t[:, q0 * 8:(q0 + qn) * 8],
                        qn * P, qn * P, rlen, elem_step=RSTRIDE,
                        single_packet=False))

                acc_t = accp.tile([P, ng_max, ACC_W], F32, tag="acc")

                for (br, g0, ngb, w, col0) in plan.batches:
                    if br != r:
                        continue
                    cols_b = ngb * w
                    Gb = G[:, col0:col0 + cols_b, :]
                    tt = wp.tile([P, cols_max * 2], F32, tag="tt")
                    at = wp.tile([P, cols_max * 2], F32, tag="at")
                    ert_b = ert[:, 0:ng_r, :]
                    if layer == 1:
                        el_ap = _apx(Gb, 16, [[rlen * w, ngb], [rlen, w],
                                              [1, 2]])
                        er_ap = _apx(ert_b, g0 * 2, [[2, ngb], [0, w],
                                                     [1, 2]])
                        t_ap = _apx(tt[:, :], 0, [[2 * w, ngb], [1, w],
                                                  [w, 2]])
                        nact = cols_b * 2
                    else:
                        el_ap = _apx(Gb, 16, [[rlen, cols_b]])
                        er_ap = _apx(ert_b, g0 * 2, [[2, ngb], [0, w]])
                        t_ap = tt[:, 0:cols_b]
                        nact = cols_b
                    nc.vector.tensor_tensor(out=t_ap, in0=el_ap,
                                            in1=er_ap, op=ALU.add)
                    # leaky_relu(t, 0.2) = max(0.2*t, t)
                    nc.vector.scalar_tensor_tensor(
                        out=tt[:, 0:nact], in0=tt[:, 0:nact], scalar=0.2,
                        in1=tt[:, 0:nact], op0=ALU.mult, op1=ALU.max)
                    nc.scalar.activation(at[:, 0:nact], tt[:, 0:nact],
                                         AF.Exp)
                    V2 = wp.tile([P, cols_max, 16], F32, tag="V2")
                    for hd in range(nh):
                        fw = 16 // nh
                        h_ap = _apx(Gb, hd * fw, [[rlen * w, ngb],
                                                  [rlen, w], [1, fw]])
                        if layer == 1:
                            a_ap = _apx(at[:, :], hd * w,
                                        [[2 * w, ngb], [1, w], [0, fw]])
                            v_ap = _apx(V2[:, :, :], hd * fw * w,
                                        [[16 * w, ngb], [1, w], [w, fw]])
                        else:
                            a_ap = _apx(at[:, :], 0,
                                        [[w, ngb], [1, w], [0, fw]])
                            v_ap = _apx(V2[:, :, :], 0,
                                        [[16 * w, ngb], [1, w], [w, fw]])
                        nc.vector.tensor_tensor(out=v_ap, in0=h_ap,
                                                in1=a_ap, op=ALU.mult)
                    vred = _apx(V2[:, :, :], 0, [[16 * w, ngb], [w, 16],
                                                 [1, w]])
                    m_ap = _apx(acc_t[:, :, :], g0 * ACC_W,
                                [[ACC_W, ngb], [1, 16]])
                    nc.vector.tensor_reduce(out=m_ap, in_=vred, axis=AX.X,
                                            op=ALU.add)
                    if layer == 1:
                        den_in = _apx(at[:, :], 0, [[2 * w, ngb], [w, 2],
                                                    [1, w]])
                        den_out = _apx(acc_t[:, :, :], g0 * ACC_W + 16,
                                       [[ACC_W, ngb], [1, 2]])
                    else:
                        den_in = _apx(at[:, :], 0, [[w, ngb], [1, w]])
                        den_out = _apx(acc_t[:, :, :], g0 * ACC_W + 16,
                                       [[ACC_W, ngb]])
                    nc.vector.tensor_reduce(out=den_out, in_=den_in,
                                            axis=AX.X, op=ALU.add)
                    if layer == 2:
                        # unused den slot: keep deterministic zeros
                        pass

                for q0 in range(0, ng_r, CALL_COLS):
                    qn = min(CALL_COLS, ng_r - q0)
                    chain(nc.gpsimd.dma_scatter_add(
                        acc_view, acc_t[:, q0:q0 + qn, :],
                        scidx_t[:, q0 * 8:(q0 + qn) * 8],
                        qn * P, qn * P, ACC_W, elem_step=RSTRIDE,
                        single_packet=False))

    edge_layer(1)

    # ---------- layer-1 finalize: h0 / el2 / er2 -> table2 ----------
    with tc.tile_pool(name=f"fin1_{rep}", bufs=1) as fin:
        accl = fin.tile([P, J, ACC_W], F32)
        nc.sync.dma_start(
            out=bass.AP(tensor=acc1, offset=c.shard_real * RSTRIDE,
                        ap=[[RSTRIDE, npad], [1, RSTRIDE]]),
            in_=zero_sb[0:npad, 0:RSTRIDE])
        nc.sync.dma_start(
            out=accl[:, :, :],
            in_=bass.AP(tensor=acc1, offset=0,
                        ap=[[RSTRIDE, P], [RSTRIDE * P, J], [1, ACC_W]]))
        dmax = fin.tile([P, J, 2], F32)
        nc.vector.tensor_scalar_max(out=dmax[:, :, :], in0=accl[:, :, 16:18],
                                    scalar1=1e-9)
        rec = fin.tile([P, J, 2], F32)
        nc.vector.reciprocal(out=rec[:, :, :], in_=dmax[:, :, :])
        h0p = fin.tile([P, J, 16], F32)
        rec_b = _apx(rec[:, :, :], 0, [[2, J], [1, 2], [0, 8]])
        nc.vector.tensor_tensor(out=h0p[:, :, :], in0=accl[:, :, 0:16],
                                in1=rec_b, op=ALU.mult)
        b1_b = _apx(T["b1_sb"][:, :], 0, [[0, J], [1, 16]])
        nc.vector.tensor_tensor(out=h0p[:, :, :], in0=h0p[:, :, :], in1=b1_b,
                                op=ALU.add)
        nc.vector.tensor_scalar_max(out=hub2[:, :, 0:16], in0=h0p[:, :, :],
                                    scalar1=0.0)
        tmp = fin.tile([P, J, 16], F32)
        vl_b = _apx(T["vl_sb"][:, :], 0, [[0, J], [1, 16]])
        nc.vector.tensor_tensor(out=tmp[:, :, :], in0=hub2[:, :, 0:16],
                                in1=vl_b, op=ALU.mult)
        nc.vector.tensor_reduce(out=hub2[:, :, 16], in_=tmp[:, :, :],
                                axis=AX.X, op=ALU.add)
        vr_b = _apx(T["vr_sb"][:, :], 0, [[0, J], [1, 16]])
        nc.vector.tensor_tensor(out=tmp[:, :, :], in0=hub2[:, :, 0:16],
                                in1=vr_b, op=ALU.mult)
        nc.vector.tensor_reduce(out=hub2[:, :, 17], in_=tmp[:, :, :],
                                axis=AX.X, op=ALU.add)
        nc.vector.memset(hub2[:, :, 18:20], 0.0)
        nc.sync.dma_start(
            out=bass.AP(tensor=tab2_loc, offset=0,
                        ap=[[ROW, P], [ROW * P, J], [1, ROW]]),
            in_=hub2[:, :, :])
        nc.sync.dma_start(
            out=bass.AP(tensor=tab2_loc, offset=c.shard_real * ROW,
                        ap=[[ROW, npad], [1, ROW]]),
            in_=T["pad2_sb"][0:npad, :])
        nc.sync.dma_start(
            out=bass.AP(tensor=er2_loc, offset=0,
                        ap=[[RSTRIDE, P], [RSTRIDE * P, J], [1, 2]]),
            in_=hub2[:, :, 17:19])

    nc.gpsimd.collective_compute(
        "AllGather", ALU.bypass, replica_groups=[list(range(NCORES))],
        ins=[tab2_loc.ap()], outs=[tab2_full.ap()])
    for sp0 in range(0, NF, 32768):
        spn = min(32768, NF - sp0)
        nc.sync.dma_start(
            out=bass.AP(tensor=gtab2, offset=sp0 * RSTRIDE,
                        ap=[[RSTRIDE, spn], [1, ROW]]),
            in_=tab2_full.ap()[sp0:sp0 + spn, :])

    edge_layer(2)

    # ---------- layer-2 finalize -> spartial ----------
    with tc.tile_pool(name=f"fin2_{rep}", bufs=1) as fin, \
         tc.tile_pool(name=f"fps_{rep}", bufs=1, space="PSUM") as fps:
        acc2l = fin.tile([P, J, 17], F32)
        nc.sync.dma_start(
            out=bass.AP(tensor=acc2, offset=c.shard_real * RSTRIDE,
                        ap=[[RSTRIDE, npad], [1, RSTRIDE]]),
            in_=zero_sb[0:npad, 0:RSTRIDE])
        nc.sync.dma_start(
            out=acc2l[:, :, :],
            in_=bass.AP(tensor=acc2, offset=0,
                        ap=[[RSTRIDE, P], [RSTRIDE * P, J], [1, 17]]))
        d2 = fin.tile([P, J], F32)
        nc.vector.tensor_scalar_max(out=d2[:, :], in0=acc2l[:, :, 16],
                                    scalar1=1e-9)
        r2 = fin.tile([P, J], F32)
        nc.vector.reciprocal(out=r2[:, :], in_=d2[:, :])
        rt = fin.tile([P, 16, J], F32)
        r2_b = _apx(r2[:, :], 0, [[1, J], [0, 16]])
        rt_ap = _apx(rt[:, :, :], 0, [[1, J], [J, 16]])
        nc.vector.tensor_tensor(out=rt_ap, in0=acc2l[:, :, 0:16], in1=r2_b,
                                op=ALU.mult)
        S_acc = fin.tile([P, 16], F32)
        nc.vector.tensor_reduce(out=S_acc[:, :], in_=rt[:, :, :], axis=AX.X,
                                op=ALU.add)
        ones = fin.tile([P, 1], F32)
        nc.vector.memset(ones[:, :], 1.0)
        sp = fps.tile([16, 1], F32, space="PSUM")
        nc.tensor.matmul(sp[:, :], S_acc[:, :], ones[:, :], start=True,
                         stop=True)
        sout = fin.tile([16, 1], F32)
        nc.vector.tensor_copy(out=sout[:, :], in_=sp[:, :])
        nc.sync.dma_start(out=T["spartial"].ap(), in_=sout[:, :])


# ---------------- host orchestration ----------------

_CACHE = {}


def _get(cfg, src0, dst0, nreps=1):
    key = (cfg.shard_real, cfg.jcount, cfg.nrounds, nreps,
           hash(src0.tobytes()), hash(dst0.tobytes()))
    if key not in _CACHE:
        plan = Plan(cfg, src0, dst0)
        nc = build_program(cfg, plan, nreps=nreps)
        _CACHE[key] = (plan, nc)
    return _CACHE[key]


def make_in_maps(cfg, plan, inputs):
    c = cfg
    s = math.sqrt(2.0 / 64.0)
    feat0 = np.asarray(inputs["feat0"], dtype=np.float32)
    W_rbf0 = np.asarray(inputs["W_rbf0"], dtype=np.float32)
    b_rbf0 = np.asarray(inputs["b_rbf0"], dtype=np.float32)
    g2c1_W = np.asarray(inputs["g2c1_W"], dtype=np.float32)
    g2c1_al = np.asarray(inputs["g2c1_al"], dtype=np.float32)
    g2c1_ar = np.asarray(inputs["g2c1_ar"], dtype=np.float32)
    g2c1_b = np.asarray(inputs["g2c1_b"], dtype=np.float32)
    g2c2_W = np.asarray(inputs["g2c2_W"], dtype=np.float32)
    g2c2_al = np.asarray(inputs["g2c2_al"], dtype=np.float32)
    g2c2_ar = np.asarray(inputs["g2c2_ar"], dtype=np.float32)

    dfeat = feat0.shape[1]
    waug = np.zeros((64, 64), dtype=np.float32)
    waug[:dfeat, :] = W_rbf0
    waug[dfeat, :] = b_rbf0 + PHASE_SHIFT
    al16 = np.zeros((16, 2), dtype=np.float32)
    ar16 = np.zeros((16, 2), dtype=np.float32)
    for hd in range(2):
        al16[hd * 8:(hd + 1) * 8, hd] = g2c1_al[hd]
        ar16[hd * 8:(hd + 1) * 8, hd] = g2c1_ar[hd]
    l20 = np.zeros((64, ROW), dtype=np.float32)
    l20[:, 0:16] = s * g2c1_W
    l20[:, 16:18] = s * (g2c1_W @ al16)
    l20[:, 18:20] = s * (g2c1_W @ ar16)
    vl = (g2c2_W @ g2c2_al[0]).astype(np.float32)
    vr = (g2c2_W @ g2c2_ar[0]).astype(np.float32)

    maps = []
    for cc in range(NCORES):
        ft = np.zeros((64, c.nloc), dtype=np.float32)
        lo = cc * c.shard_real
        ft[:dfeat, :c.shard_real] = feat0[lo:lo + c.shard_real].T
        ft[dfeat, :] = 1.0
        maps.append({
            "featT": ft,
            "waug": waug,
            "l20": l20,
            "b1ext": np.tile(g2c1_b.reshape(1, 16), (P, 1)),
            "vlext": np.tile(vl.reshape(1, 16), (P, 1)),
            "vrext": np.tile(vr.reshape(1, 16), (P, 1)),
            "gidx": plan.gidx_cat[cc],
            "eridx": plan.eridx_cat[cc],
            "scidx": plan.scidx_cat[cc],
        })
    return maps


def host_tail(cfg, inputs, spartials):
    S = np.zeros(16, dtype=np.float64)
    for cc in range(NCORES):
        S += spartials[cc][:, 0].astype(np.float64)
    n_nodes = NCORES * cfg.shard_real
    W2 = np.asarray(inputs["g2c2_W"], dtype=np.float64)
    b2 = np.asarray(inputs["g2c2_b"], dtype=np.float64)
    mean = (S @ W2) / n_nodes + b2
    h = np.maximum(mean, 0.0)
    h = np.maximum(
        h @ np.asarray(inputs["fc1_w"], dtype=np.float64).T
        + np.asarray(inputs["fc1_b"], dtype=np.float64), 0.0)
    out = (h @ np.asarray(inputs["out_w"], dtype=np.float64).T
           + np.asarray(inputs["out_b"], dtype=np.float64))
    return out.astype(np.float32).reshape(1)


def kernel(**inputs):
    cfg = FULL
    src0 = np.asarray(inputs["src0"])
    dst0 = np.asarray(inputs["dst0"])
    plan, nc = _get(cfg, src0, dst0)
    in_maps = make_in_maps(cfg, plan, inputs)
    res = bass_utils.run_bass_kernel_spmd(nc, in_maps,
                                          core_ids=list(range(NCORES)))
    return host_tail(cfg, inputs, [res.results[cc]["spartial"]
                                   for cc in range(NCORES)])


# revision 17
# speedup vs baseline: 1.5883x; 1.4546x over previous
"""Trainium2 Bass kernel for nn_GAT_78151224918248 (gnn_message_passing).

Only the g0 branch of the reference is live (the g1 branch's output `ef` is
discarded), so the kernel computes
    nf0  = sqrt(2/64)*cos(feat0 @ W_rbf0 + b_rbf0)
    h0   = relu(gat_conv(nf0, g2c1_*))        # H=2, F=8
    out2 = gat_conv(h0, g2c2_*)               # H=1, F=64
    y    = MLP(relu(mean(out2, axis=0)))
and the final scalar is assembled on the host from per-core [16] partial sums
(g2c2_W is pulled out of the segment sums by linearity, so only 16-wide node
messages are aggregated on-device).

Distribution: nodes are sharded 25000/core across 8 NeuronCores (dst-major
edge sharding). Per layer, a 20-float node-record table is AllGathered; edges
are processed in 7 "rounds" by source-row range (to fit dma_gather's int16
indices), each round with a degree-sorted slot grid so all per-edge math is
plain broadcast/reduce vector work; per-round node partial sums are folded
into a DRAM accumulator with dma_scatter_add. Gather/scatter calls are one
per round (SWDGE streams descriptors through the ring with backpressure).
"""
import sys

for _p in ("/opt/trn_rl_repo", "/opt/pypackages"):
    if _p not in sys.path:
        sys.path.insert(0, _p)

import math
import numpy as np

import concourse.bass as bass
import concourse.bacc as bacc
import concourse.tile as tile
from concourse import mybir
from concourse import bass_utils

F32 = mybir.dt.float32
I16 = mybir.dt.int16
AF = mybir.ActivationFunctionType
ALU = mybir.AluOpType
AX = mybir.AxisListType

NCORES = 8
P = 128
TWO_PI = 2.0 * math.pi
PHASE_SHIFT = math.pi / 2.0
MAGIC = 12582912.0  # 1.5*2^23: fp32 add/sub rounds to nearest int

ROW = 20        # floats per table-row record
RSTRIDE = 64    # 256B stride of gatherable tables
ACC_W = 18      # floats scatter-added per node


class Cfg:
    def __init__(self, shard_real, jcount, nrounds):
        self.shard_real = shard_real
        self.jcount = jcount
        self.nloc = P * jcount
        assert shard_real <= self.nloc and shard_real >= (jcount - 1) * P
        self.nfull = NCORES * self.nloc
        self.nrounds = nrounds
        assert self.nfull % nrounds == 0
        self.rng_rows = self.nfull // nrounds
        assert self.rng_rows <= 32767


FULL = Cfg(shard_real=25000, jcount=196, nrounds=7)


def _wrap_idx16(vals):
    """[n] ints -> [128, ceil(n/16)] int16 SWDGE idx layout (idx i at
    [i%16, i//16], replicated to the 8 16-partition groups)."""
    n = len(vals)
    w = (n + 15) // 16
    pad = np.zeros(w * 16, dtype=np.int64)
    pad[:n] = vals
    a = np.zeros((P, w), dtype=np.int16)
    blk = pad.astype(np.int16).reshape(w, 16).T
    for g in range(8):
        a[g * 16:(g + 1) * 16, :] = blk
    return a


class Plan:
    """Host-side graph preprocessing shared by both layers."""

    def __init__(self, cfg: Cfg, src: np.ndarray, dst: np.ndarray):
        c = cfg
        self.cfg = c
        n_nodes = NCORES * c.shard_real
        src = src.astype(np.int64)
        dst = dst.astype(np.int64)
        assert src.min() >= 0 and src.max() < n_nodes
        assert dst.min() >= 0 and dst.max() < n_nodes

        core_of = dst // c.shard_real
        rows_of = (src // c.shard_real) * c.nloc + (src % c.shard_real)
        dloc = dst % c.shard_real
        rnd_of = rows_of // c.rng_rows

        # a pad table row inside every round's range (gather dummy target)
        pad_rows = np.concatenate(
            [cc * c.nloc + np.arange(c.shard_real, c.nloc) for cc in range(NCORES)])
        self.dummy = np.zeros(c.nrounds, dtype=np.int64)
        for r in range(c.nrounds):
            in_r = pad_rows[(pad_rows >= r * c.rng_rows)
                            & (pad_rows < (r + 1) * c.rng_rows)]
            assert len(in_r) > 0, f"no pad row available for round {r}"
            self.dummy[r] = in_r[0]

        # per (core, round) degree tables and node orders
        deg = np.zeros((NCORES, c.nrounds, c.nloc), dtype=np.int64)
        np.add.at(deg, (core_of, rnd_of, dloc), 1)
        orders = [[None] * c.nrounds for _ in range(NCORES)]
        for cc in range(NCORES):
            for r in range(c.nrounds):
                d = deg[cc, r]
                act = np.nonzero(d)[0]
                orders[cc][r] = act[np.argsort(-d[act], kind="stable")]

        # group templates shared across cores
        self.ng = np.zeros(c.nrounds, dtype=np.int64)
        self.widths = []
        for r in range(c.nrounds):
            ng_r = max((len(orders[cc][r]) + P - 1) // P for cc in range(NCORES))
            w_r = np.zeros(max(ng_r, 1), dtype=np.int64)[:ng_r]
            for cc in range(NCORES):
                o = orders[cc][r]
                if len(o) == 0:
                    continue
                ds = deg[cc, r][o]
                padded = np.zeros(ng_r * P, dtype=np.int64)
                padded[:len(ds)] = ds
                w_r = np.maximum(w_r, padded.reshape(ng_r, P).max(axis=1))
            self.ng[r] = ng_r
            self.widths.append(w_r)

        # batches: (round, g0, ngb, w, col0) = maximal equal-width group runs
        self.batches = []
        self.cols = np.zeros(c.nrounds, dtype=np.int64)
        for r in range(c.nrounds):
            w_r = self.widths[r]
            col = 0
            g = 0
            while g < len(w_r):
                w = int(w_r[g])
                g2 = g
                while g2 < len(w_r) and int(w_r[g2]) == w:
                    g2 += 1
                self.batches.append((r, g, g2 - g, w, col))
                col += (g2 - g) * w
                g = g2
            self.cols[r] = col

        # per-core index arrays
        trash = c.shard_real  # local pad row for scatter padding
        self.gidx_cat, self.eridx_cat, self.scidx_cat = [], [], []
        for cc in range(NCORES):
            g_parts, er_parts, sc_parts = [], [], []
            for r in range(c.nrounds):
                ng_r = int(self.ng[r])
                w_r = self.widths[r]
                cols_r = int(self.cols[r])
                o = orders[cc][r]
                nact = len(o)
                gvals = np.full(cols_r * P, self.dummy[r], dtype=np.int64)
                ervals = np.zeros(ng_r * P, dtype=np.int64)
                scvals = np.full(ng_r * P, trash, dtype=np.int64)
                if nact:
                    ervals[:nact] = o
                    scvals[:nact] = o
                    # edges of (cc, r) sorted by node position
                    m = (core_of == cc) & (rnd_of == r)
                    ed, er_rows = dloc[m], rows_of[m]
                    pos_of = np.full(c.nloc, -1, dtype=np.int64)
                    pos_of[o] = np.arange(nact)
                    pe = pos_of[ed]
                    si = np.argsort(pe, kind="stable")
                    pe, er_rows = pe[si], er_rows[si]
                    # k = intra-node running index
                    firsts = np.searchsorted(pe, np.arange(nact))
                    k = np.arange(len(pe)) - firsts[pe]
                    col0_of_g = np.concatenate([[0], np.cumsum(w_r)])[:-1]
                    gg, pp = pe // P, pe % P
                    slot = (col0_of_g[gg] + k) * P + pp
                    gvals[slot] = er_rows
                g_parts.append(_wrap_idx16(gvals - r * c.rng_rows))
                er_parts.append(_wrap_idx16(ervals))
                sc_parts.append(_wrap_idx16(scvals))
            self.gidx_cat.append(np.concatenate(g_parts, axis=1))
            self.eridx_cat.append(np.concatenate(er_parts, axis=1))
            self.scidx_cat.append(np.concatenate(sc_parts, axis=1))
        self.gidx_off = np.concatenate(
            [[0], np.cumsum([_wrap_idx16(np.zeros(int(self.cols[r]) * P)).shape[1]
                             for r in range(c.nrounds)])])
        self.eridx_off = np.concatenate(
            [[0], np.cumsum([int(self.ng[r]) * 8 for r in range(c.nrounds)])])


def patch_dma_gather():
    import inspect
    import textwrap
    b = bass
    if getattr(b.BassGpSimd.dma_gather, "_flex_patched", False):
        return
    src = textwrap.dedent(inspect.getsource(b.BassGpSimd.dma_gather))
    bad = ("assert (\n        elem_size_bytes > 0 and elem_size_bytes % 256 == 0\n"
           "    )  # transpose restriction")
    assert bad in src, "dma_gather source changed; fix patch"
    src = src.replace(bad, "assert elem_size_bytes > 0")
    ns = dict(vars(b))
    exec(src, ns)
    ns["dma_gather"]._flex_patched = True
    b.BassGpSimd.dma_gather = ns["dma_gather"]


def _apx(base_ap, extra_off, dims):
    """New AP on the same tensor: keep partition dim, replace free dims."""
    return bass.AP(tensor=base_ap.tensor, offset=base_ap.offset + extra_off,
                   ap=[list(base_ap.ap[0])] + [list(d) for d in dims])


def build_program(cfg: Cfg, plan: Plan, nreps: int = 1, probe: str = ""):
    patch_dma_gather()
    c = cfg
    J, NL, NF = c.jcount, c.nloc, c.nfull
    ng_max = int(max(plan.ng))
    cols_max = int(max(plan.cols))
    nqueues = 1 if "q1" in probe else (2 if "q2" in probe else 4)
    nc = bacc.Bacc("TRN2", target_bir_lowering=False, debug=False,
                   num_devices=NCORES, dynamic_dma_scratch_size=65536,
                   num_swdge_queues=nqueues)
    nc._kernel_nqueues = nqueues

    featT = nc.dram_tensor("featT", [64, NL], F32, kind="ExternalInput")
    waug = nc.dram_tensor("waug", [64, 64], F32, kind="ExternalInput")
    l20 = nc.dram_tensor("l20", [64, ROW], F32, kind="ExternalInput")
    b1ext = nc.dram_tensor("b1ext", [P, 16], F32, kind="ExternalInput")
    vlext = nc.dram_tensor("vlext", [P, 16], F32, kind="ExternalInput")
    vrext = nc.dram_tensor("vrext", [P, 16], F32, kind="ExternalInput")
    gidx_h = nc.dram_tensor("gidx", [P, int(plan.gidx_off[-1])], I16,
                            kind="ExternalInput")
    eridx_h = nc.dram_tensor("eridx", [P, int(plan.eridx_off[-1])], I16,
                             kind="ExternalInput")
    scidx_h = nc.dram_tensor("scidx", [P, int(plan.eridx_off[-1])], I16,
                             kind="ExternalInput")
    spartial = nc.dram_tensor("spartial", [16, 1], F32, kind="ExternalOutput")

    tab1_loc = nc.dram_tensor("tab1_loc", [NL, RSTRIDE], F32)
    tab2_loc = nc.dram_tensor("tab2_loc", [NL, RSTRIDE], F32)
    gtab1 = nc.dram_tensor("gtab1", [NF, RSTRIDE], F32, addr_space="Shared")
    gtab2 = nc.dram_tensor("gtab2", [NF, RSTRIDE], F32, addr_space="Shared")
    acc1 = nc.dram_tensor("acc1", [NL, RSTRIDE], F32)
    acc2 = nc.dram_tensor("acc2", [NL, RSTRIDE], F32)

    pad_p0 = c.shard_real - (J - 1) * P  # pads are (p >= pad_p0, j == J-1)
    npad = NL - c.shard_real
    assert 0 < npad <= P

    with tile.TileContext(nc) as tc:
        with tc.tile_pool(name="persist", bufs=1) as pers:
            waug_sb = pers.tile([64, 64], F32)
            nc.sync.dma_start(out=waug_sb[:, :], in_=waug.ap())
            l20_sb = pers.tile([P, ROW], F32)
            nc.sync.dma_start(out=l20_sb[0:64, :], in_=l20.ap())
            nc.sync.dma_start(out=l20_sb[64:128, :], in_=l20.ap())
            b1_sb = pers.tile([P, 16], F32)
            nc.sync.dma_start(out=b1_sb[:, :], in_=b1ext.ap())
            vl_sb = pers.tile([P, 16], F32)
            nc.sync.dma_start(out=vl_sb[:, :], in_=vlext.ap())
            vr_sb = pers.tile([P, 16], F32)
            nc.sync.dma_start(out=vr_sb[:, :], in_=vrext.ap())
            zero_sb = pers.tile([P, 2048], F32)
            nc.vector.memset(zero_sb[:, :], 0.0)
            # pad-row record for layer1/layer2 tables: zeros except el=-1e30
            pad1_sb = pers.tile([P, ROW], F32)
            nc.vector.memset(pad1_sb[:, :], 0.0)
            nc.vector.memset(pad1_sb[:, 16:18], -1.0e30)
            pad2_sb = pers.tile([P, ROW], F32)
            nc.vector.memset(pad2_sb[:, :], 0.0)
            nc.vector.memset(pad2_sb[:, 16:17], -1.0e30)
            hub1 = pers.tile([P, J, ROW], F32)
            hub2 = pers.tile([P, J, ROW], F32)

            T = dict(featT=featT, waug_sb=waug_sb, l20_sb=l20_sb,
                     b1_sb=b1_sb, vl_sb=vl_sb, vr_sb=vr_sb,
                     zero_sb=zero_sb, pad1_sb=pad1_sb,
                     pad2_sb=pad2_sb, hub1=hub1, hub2=hub2,
                     gidx_h=gidx_h, eridx_h=eridx_h, scidx_h=scidx_h,
                     tab1_loc=tab1_loc, tab2_loc=tab2_loc,
                     gtab1=gtab1, gtab2=gtab2, acc1=acc1, acc2=acc2,
                     spartial=spartial)
            T["probe"] = probe
            for rep in range(nreps):
                body_once(nc, tc, c, plan, rep, T, ng_max, cols_max)

    nc.compile()
    return nc


CALL_COLS = 28  # 3584 indices per SWDGE call (ring is 4096 descriptors)
STAGE_J = 28    # J-chunk per staged 64-wide table write


def store_table(nc, tc, c, rep, layer, hub, tab_loc, pad_sb):
    """hub [P, J, ROW] -> tab_loc [NL, RSTRIDE] (contiguous 256B-row writes;
    pad columns carry garbage, never read). Row index of node (p, j) is
    j*128 + p, matching the gather index construction."""
    J = c.jcount
    npad = c.nloc - c.shard_real
    with tc.tile_pool(name=f"st{layer}_{rep}", bufs=2) as stp:
        for j0 in range(0, J, STAGE_J):
            jn = min(STAGE_J, J - j0)
            st = stp.tile([P, STAGE_J, RSTRIDE], F32, tag="st")
            nc.vector.tensor_copy(out=st[:, 0:jn, 0:ROW],
                                  in_=hub[:, j0:j0 + jn, :])
            nc.sync.dma_start(
                out=bass.AP(tensor=tab_loc, offset=j0 * P * RSTRIDE,
                            ap=[[RSTRIDE, P], [RSTRIDE * P, jn], [1, RSTRIDE]]),
                in_=st[:, 0:jn, :])
    nc.sync.dma_start(
        out=bass.AP(tensor=tab_loc, offset=c.shard_real * RSTRIDE,
                    ap=[[RSTRIDE, npad], [1, ROW]]),
        in_=pad_sb[0:npad, :])


def body_once(nc, tc, c, plan, rep, T, ng_max, cols_max):
    call_cols = 7 if "smallcall" in T["probe"] else CALL_COLS
    J, NL, NF = c.jcount, c.nloc, c.nfull
    npad = NL - c.shard_real

    nq = getattr(nc, "_kernel_nqueues", 1)
    if nq > 1:
        call_cols = min(call_cols, 14)
    qchains = T.setdefault("swdge_qchains", [[] for _ in range(nq)])
    qrr = T.setdefault("swdge_qrr", [0])

    def next_q():
        q = qrr[0] % nq
        qrr[0] += 1
        return q

    def chain(inst, q):
        ch = qchains[q]
        if len(ch) >= 2:
            bass._add_dep_helper(inst.ins, ch[-2].ins, sync=True,
                                 reason="swdge ring throttle")
        ch.append(inst)
        return inst
    featT, zero_sb = T["featT"], T["zero_sb"]
    hub1, hub2 = T["hub1"], T["hub2"]
    tab1_loc, tab2_loc = T["tab1_loc"], T["tab2_loc"]
    gtab1, gtab2 = T["gtab1"], T["gtab2"]
    acc1, acc2 = T["acc1"], T["acc2"]

    # ---------- phase 0: RBF + layer-1 node records ----------
    half = NL // 2
    assert half % P == 0
    blk = 1792 if half % 1792 == 0 else P
    nblk = half // blk
    jt_per_blk = blk // P
    with tc.tile_pool(name=f"ph0_{rep}", bufs=3) as ph0, \
         tc.tile_pool(name=f"nf0p_{rep}", bufs=2) as nf0p, \
         tc.tile_pool(name=f"ph0ps_{rep}", bufs=2, space="PSUM") as ph0ps, \
         tc.tile_pool(name=f"zhps_{rep}", bufs=4, space="PSUM") as zhps:
        for b in range(nblk):
            c0 = b * blk
            nf0_t = nf0p.tile([P, blk], F32, tag="nf0")
            for ch0 in range(0, blk, 512):
                cw = min(512, blk - ch0)
                ft = ph0.tile([64, 512], F32, tag="ft")
                nc.sync.dma_start(out=ft[:, :cw],
                                  in_=featT.ap()[:, c0 + ch0:c0 + ch0 + cw])
                ft2 = ph0.tile([64, 512], F32, tag="ft2")
                nc.sync.dma_start(
                    out=ft2[:, :cw],
                    in_=featT.ap()[:, half + c0 + ch0:half + c0 + ch0 + cw])
                ps = ph0ps.tile([P, 512], F32, space="PSUM", tag="ps")
                nc.tensor.matmul(ps[0:64, :cw], T["waug_sb"][:, :], ft[:, :cw],
                                 start=True, stop=True)
                nc.tensor.matmul(ps[64:128, :cw], T["waug_sb"][:, :],
                                 ft2[:, :cw], start=True, stop=True)
                wt = ph0.tile([P, 512], F32, tag="wt")
                kt = ph0.tile([P, 512], F32, tag="kt")
                # k = round(z / 2pi) via the fp32 magic constant
                nc.vector.tensor_scalar(out=kt[:, :cw], in0=ps[:, :cw],
                                        scalar1=1.0 / TWO_PI, scalar2=MAGIC,
                                        op0=ALU.mult, op1=ALU.add)
                nc.vector.tensor_scalar_add(out=kt[:, :cw], in0=kt[:, :cw],
                                            scalar1=-MAGIC)
                # w = z - k*2pi, clamped into the Sin LUT domain
                nc.vector.scalar_tensor_tensor(
                    out=wt[:, :cw], in0=kt[:, :cw], scalar=-TWO_PI,
                    in1=ps[:, :cw], op0=ALU.mult, op1=ALU.add)
                nc.vector.tensor_scalar(out=wt[:, :cw], in0=wt[:, :cw],
                                        scalar1=math.pi * 0.9999999,
                                        scalar2=-math.pi * 0.9999999,
                                        op0=ALU.min, op1=ALU.max)
                nc.scalar.activation(nf0_t[:, ch0:ch0 + cw], wt[:, :cw],
                                     AF.Sin)
            for hs in range(2):
                zb = zhps.tile([P, jt_per_blk, ROW], F32, space="PSUM",
                               tag="zb")
                for jj in range(jt_per_blk):
                    nc.tensor.matmul(
                        zb[:, jj, :],
                        nf0_t[hs * 64:(hs + 1) * 64, jj * P:(jj + 1) * P],
                        T["l20_sb"][hs * 64:(hs + 1) * 64, :],
                        start=True, stop=True)
                jbase = (hs * half + c0) // P
                nc.scalar.activation(hub1[:, jbase:jbase + jt_per_blk, :],
                                     zb[:, :, :], AF.Identity)
    store_table(nc, tc, c, rep, 1, hub1, tab1_loc, T["pad1_sb"])

    if "noag" in T["probe"]:
        for cc8 in range(NCORES):
            nc.sync.dma_start(out=gtab1.ap()[cc8 * NL:(cc8 + 1) * NL, :],
                              in_=tab1_loc.ap())
    else:
        nc.gpsimd.collective_compute(
            "AllGather", ALU.bypass, replica_groups=[list(range(NCORES))],
            ins=[tab1_loc.ap()], outs=[gtab1.ap()])

    # zero accumulators (runs alongside the collective)
    for a in (acc1, acc2):
        tot = NL * RSTRIDE
        step = P * 2048
        off = 0
        while off < tot:
            sz = min(step, tot - off)
            assert sz % P == 0
            q = sz // P
            v = bass.AP(tensor=a, offset=off, ap=[[q, P], [1, q]])
            nc.sync.dma_start(out=v, in_=zero_sb[:, :q])
            off += sz

    # ---------- edge stage ----------
    def edge_layer(layer):
        gtab = gtab1 if layer == 1 else gtab2
        tab_loc = tab1_loc if layer == 1 else tab2_loc
        acc = acc1 if layer == 1 else acc2
        rlen = 18 if layer == 1 else 17
        nh = 2 if layer == 1 else 1
        er_view = bass.AP(tensor=tab_loc, offset=18 if layer == 1 else 17,
                          ap=[[RSTRIDE, NL], [1, 2]])
        acc_view = bass.AP(tensor=acc, offset=0,
                           ap=[[RSTRIDE, NL], [1, ACC_W]])
        with tc.tile_pool(name=f"l{layer}idx_{rep}", bufs=2) as idxp, \
             tc.tile_pool(name=f"l{layer}g_{rep}", bufs=2) as gp, \
             tc.tile_pool(name=f"l{layer}w_{rep}", bufs=2) as wp, \
             tc.tile_pool(name=f"l{layer}acc_{rep}", bufs=2) as accp:
            for r in range(c.nrounds):
                ng_r = int(plan.ng[r])
                cols_r = int(plan.cols[r])
                if ng_r == 0:
                    continue
                gt_view = bass.AP(
                    tensor=gtab, offset=r * c.rng_rows * RSTRIDE,
                    ap=[[RSTRIDE, c.rng_rows], [1, rlen]])
                gi0 = int(plan.gidx_off[r])
                ei0 = int(plan.eridx_off[r])
                gw = (cols_r * P + 15) // 16
                gidx_t = idxp.tile([P, (cols_max * P + 15) // 16], I16,
                                   tag="gidx")
                nc.sync.dma_start(out=gidx_t[:, :gw],
                                  in_=T["gidx_h"].ap()[:, gi0:gi0 + gw])
                eridx_t = idxp.tile([P, ng_max * 8], I16, tag="eridx")
                nc.sync.dma_start(out=eridx_t[:, :ng_r * 8],
                                  in_=T["eridx_h"].ap()[:, ei0:ei0 + ng_r * 8])
                scidx_t = idxp.tile([P, ng_max * 8], I16, tag="scidx")
                nc.sync.dma_start(out=scidx_t[:, :ng_r * 8],
                                  in_=T["scidx_h"].ap()[:, ei0:ei0 + ng_r * 8])

                ert = gp.tile([P, ng_max, 2], F32, tag="ert")
                for q0 in range(0, ng_r, call_cols):
                    qn = min(call_cols, ng_r - q0)
                    q = next_q()
                    chain(nc.gpsimd.dma_gather(
                        ert[:, q0:q0 + qn, :], er_view,
                        eridx_t[:, q0 * 8:(q0 + qn) * 8],
                        qn * P, qn * P, 2, elem_step=RSTRIDE,
                        single_packet=False, queue_num=q), q)

                G = gp.tile([P, cols_max, rlen], F32, tag="G")
                for q0 in range(0, cols_r, call_cols):
                    qn = min(call_cols, cols_r - q0)
                    q = next_q()
                    chain(nc.gpsimd.dma_gather(
                        G[:, q0:q0 + qn, :], gt_view,
                        gidx_t[:, q0 * 8:(q0 + qn) * 8],
                        qn * P, qn * P, rlen, elem_step=RSTRIDE,
                        single_packet=False, queue_num=q), q)

                acc_t = accp.tile([P, ng_max, ACC_W], F32, tag="acc")

                for (br, g0, ngb, w, col0) in plan.batches:
                    if br != r:
                        continue
                    cols_b = ngb * w
                    Gb = G[:, col0:col0 + cols_b, :]
                    tt = wp.tile([P, cols_max * 2], F32, tag="tt")
                    at = wp.tile([P, cols_max * 2], F32, tag="at")
                    ert_b = ert[:, 0:ng_r, :]
                    if layer == 1:
                        el_ap = _apx(Gb, 16, [[rlen * w, ngb], [rlen, w],
                                              [1, 2]])
                        er_ap = _apx(ert_b, g0 * 2, [[2, ngb], [0, w],
                                                     [1, 2]])
                        t_ap = _apx(tt[:, :], 0, [[2 * w, ngb], [1, w],
                                                  [w, 2]])
                        nact = cols_b * 2
                    else:
                        el_ap = _apx(Gb, 16, [[rlen, cols_b]])
                        er_ap = _apx(ert_b, g0 * 2, [[2, ngb], [0, w]])
                        t_ap = tt[:, 0:cols_b]
                        nact = cols_b
                    nc.vector.tensor_tensor(out=t_ap, in0=el_ap,
                                            in1=er_ap, op=ALU.add)
                    # leaky_relu(t, 0.2) = max(0.2*t, t)
                    nc.vector.scalar_tensor_tensor(
                        out=tt[:, 0:nact], in0=tt[:, 0:nact], scalar=0.2,
                        in1=tt[:, 0:nact], op0=ALU.mult, op1=ALU.max)
                    nc.scalar.activation(at[:, 0:nact], tt[:, 0:nact],
                                         AF.Exp)
                    V2 = wp.tile([P, cols_max, 16], F32, tag="V2")
                    for hd in range(nh):
                        fw = 16 // nh
                        h_ap = _apx(Gb, hd * fw, [[rlen * w, ngb],
                                                  [rlen, w], [1, fw]])
                        if layer == 1:
                            a_ap = _apx(at[:, :], hd * w,
                                        [[2 * w, ngb], [1, w], [0, fw]])
                            v_ap = _apx(V2[:, :, :], hd * fw * w,
                                        [[16 * w, ngb], [1, w], [w, fw]])
                        else:
                            a_ap = _apx(at[:, :], 0,
                                        [[w, ngb], [1, w], [0, fw]])
                            v_ap = _apx(V2[:, :, :], 0,
                                        [[16 * w, ngb], [1, w], [w, fw]])
                        nc.vector.tensor_tensor(out=v_ap, in0=h_ap,
                                                in1=a_ap, op=ALU.mult)
                    vred = _apx(V2[:, :, :], 0, [[16 * w, ngb], [w, 16],
                                                 [1, w]])
                    m_ap = _apx(acc_t[:, :, :], g0 * ACC_W,
                                [[ACC_W, ngb], [1, 16]])
                    nc.vector.tensor_reduce(out=m_ap, in_=vred, axis=AX.X,
                                            op=ALU.add)
                    if layer == 1:
                        den_in = _apx(at[:, :], 0, [[2 * w, ngb], [w, 2],
                                                    [1, w]])
                        den_out = _apx(acc_t[:, :, :], g0 * ACC_W + 16,
                                       [[ACC_W, ngb], [1, 2]])
                    else:
                        den_in = _apx(at[:, :], 0, [[w, ngb], [1, w]])
                        den_out = _apx(acc_t[:, :, :], g0 * ACC_W + 16,
                                       [[ACC_W, ngb]])
                    nc.vector.tensor_reduce(out=den_out, in_=den_in,
                                            axis=AX.X, op=ALU.add)
                    if layer == 2:
                        # unused den slot: keep deterministic zeros
                        pass

                for q0 in range(0, ng_r, call_cols):
                    qn = min(call_cols, ng_r - q0)
                    q = next_q()
                    chain(nc.gpsimd.dma_scatter_add(
                        acc_view, acc_t[:, q0:q0 + qn, :],
                        scidx_t[:, q0 * 8:(q0 + qn) * 8],
                        qn * P, qn * P, ACC_W, elem_step=RSTRIDE,
                        single_packet=False, queue_num=q), q)

    if "noedge" not in T["probe"]:
        edge_layer(1)

    # ---------- layer-1 finalize: h0 / el2 / er2 -> table2 ----------
    with tc.tile_pool(name=f"fin1_{rep}", bufs=1) as fin:
        accl = fin.tile([P, J, ACC_W], F32)
        nc.sync.dma_start(
            out=bass.AP(tensor=acc1, offset=c.shard_real * RSTRIDE,
                        ap=[[RSTRIDE, npad], [1, RSTRIDE]]),
            in_=zero_sb[0:npad, 0:RSTRIDE])
        nc.sync.dma_start(
            out=accl[:, :, :],
            in_=bass.AP(tensor=acc1, offset=0,
                        ap=[[RSTRIDE, P], [RSTRIDE * P, J], [1, ACC_W]]))
        dmax = fin.tile([P, J, 2], F32)
        nc.vector.tensor_scalar_max(out=dmax[:, :, :], in0=accl[:, :, 16:18],
                                    scalar1=1e-9)
        rec = fin.tile([P, J, 2], F32)
        nc.vector.reciprocal(out=rec[:, :, :], in_=dmax[:, :, :])
        h0p = fin.tile([P, J, 16], F32)
        rec_b = _apx(rec[:, :, :], 0, [[2, J], [1, 2], [0, 8]])
        nc.vector.tensor_tensor(out=h0p[:, :, :], in0=accl[:, :, 0:16],
                                in1=rec_b, op=ALU.mult)
        b1_b = _apx(T["b1_sb"][:, :], 0, [[0, J], [1, 16]])
        nc.vector.tensor_tensor(out=h0p[:, :, :], in0=h0p[:, :, :], in1=b1_b,
                                op=ALU.add)
        nc.vector.tensor_scalar_max(out=hub2[:, :, 0:16], in0=h0p[:, :, :],
                                    scalar1=0.0)
        tmp = fin.tile([P, J, 16], F32)
        vl_b = _apx(T["vl_sb"][:, :], 0, [[0, J], [1, 16]])
        nc.vector.tensor_tensor(out=tmp[:, :, :], in0=hub2[:, :, 0:16],
                                in1=vl_b, op=ALU.mult)
        nc.vector.tensor_reduce(out=hub2[:, :, 16], in_=tmp[:, :, :],
                                axis=AX.X, op=ALU.add)
        vr_b = _apx(T["vr_sb"][:, :], 0, [[0, J], [1, 16]])
        nc.vector.tensor_tensor(out=tmp[:, :, :], in0=hub2[:, :, 0:16],
                                in1=vr_b, op=ALU.mult)
        nc.vector.tensor_reduce(out=hub2[:, :, 17], in_=tmp[:, :, :],
                                axis=AX.X, op=ALU.add)
        nc.vector.memset(hub2[:, :, 18:20], 0.0)
    store_table(nc, tc, c, rep, 2, hub2, tab2_loc, T["pad2_sb"])

    if "noag" in T["probe"]:
        for cc8 in range(NCORES):
            nc.sync.dma_start(out=gtab2.ap()[cc8 * NL:(cc8 + 1) * NL, :],
                              in_=tab2_loc.ap())
    else:
        nc.gpsimd.collective_compute(
            "AllGather", ALU.bypass, replica_groups=[list(range(NCORES))],
            ins=[tab2_loc.ap()], outs=[gtab2.ap()])

    if "noedge" not in T["probe"]:
        edge_layer(2)

    # ---------- layer-2 finalize -> spartial ----------
    with tc.tile_pool(name=f"fin2_{rep}", bufs=1) as fin, \
         tc.tile_pool(name=f"fps_{rep}", bufs=1, space="PSUM") as fps:
        acc2l = fin.tile([P, J, 17], F32)
        nc.sync.dma_start(
            out=bass.AP(tensor=acc2, offset=c.shard_real * RSTRIDE,
                        ap=[[RSTRIDE, npad], [1, RSTRIDE]]),
            in_=zero_sb[0:npad, 0:RSTRIDE])
        nc.sync.dma_start(
            out=acc2l[:, :, :],
            in_=bass.AP(tensor=acc2, offset=0,
                        ap=[[RSTRIDE, P], [RSTRIDE * P, J], [1, 17]]))
        d2 = fin.tile([P, J], F32)
        nc.vector.tensor_scalar_max(out=d2[:, :], in0=acc2l[:, :, 16],
                                    scalar1=1e-9)
        r2 = fin.tile([P, J], F32)
        nc.vector.reciprocal(out=r2[:, :], in_=d2[:, :])
        rt = fin.tile([P, 16, J], F32)
        r2_b = _apx(r2[:, :], 0, [[1, J], [0, 16]])
        rt_ap = _apx(rt[:, :, :], 0, [[1, J], [J, 16]])
        nc.vector.tensor_tensor(out=rt_ap, in0=acc2l[:, :, 0:16], in1=r2_b,
                                op=ALU.mult)
        S_acc = fin.tile([P, 16], F32)
        nc.vector.tensor_reduce(out=S_acc[:, :], in_=rt[:, :, :], axis=AX.X,
                                op=ALU.add)
        ones = fin.tile([P, 1], F32)
        nc.vector.memset(ones[:, :], 1.0)
        sp = fps.tile([16, 1], F32, space="PSUM")
        nc.tensor.matmul(sp[:, :], S_acc[:, :], ones[:, :], start=True,
                         stop=True)
        sout = fin.tile([16, 1], F32)
        nc.vector.tensor_copy(out=sout[:, :], in_=sp[:, :])
        nc.sync.dma_start(out=T["spartial"].ap(), in_=sout[:, :])


# ---------------- host orchestration ----------------

_CACHE = {}


def _get(cfg, src0, dst0, nreps=1, probe=""):
    key = (cfg.shard_real, cfg.jcount, cfg.nrounds, nreps, probe,
           hash(src0.tobytes()), hash(dst0.tobytes()))
    if key not in _CACHE:
        plan = Plan(cfg, src0, dst0)
        nc = build_program(cfg, plan, nreps=nreps, probe=probe)
        _CACHE[key] = (plan, nc)
    return _CACHE[key]


def make_in_maps(cfg, plan, inputs):
    c = cfg
    s = math.sqrt(2.0 / 64.0)
    feat0 = np.asarray(inputs["feat0"], dtype=np.float32)
    W_rbf0 = np.asarray(inputs["W_rbf0"], dtype=np.float32)
    b_rbf0 = np.asarray(inputs["b_rbf0"], dtype=np.float32)
    g2c1_W = np.asarray(inputs["g2c1_W"], dtype=np.float32)
    g2c1_al = np.asarray(inputs["g2c1_al"], dtype=np.float32)
    g2c1_ar = np.asarray(inputs["g2c1_ar"], dtype=np.float32)
    g2c1_b = np.asarray(inputs["g2c1_b"], dtype=np.float32)
    g2c2_W = np.asarray(inputs["g2c2_W"], dtype=np.float32)
    g2c2_al = np.asarray(inputs["g2c2_al"], dtype=np.float32)
    g2c2_ar = np.asarray(inputs["g2c2_ar"], dtype=np.float32)

    dfeat = feat0.shape[1]
    waug = np.zeros((64, 64), dtype=np.float32)
    waug[:dfeat, :] = W_rbf0
    waug[dfeat, :] = b_rbf0 + PHASE_SHIFT
    al16 = np.zeros((16, 2), dtype=np.float32)
    ar16 = np.zeros((16, 2), dtype=np.float32)
    for hd in range(2):
        al16[hd * 8:(hd + 1) * 8, hd] = g2c1_al[hd]
        ar16[hd * 8:(hd + 1) * 8, hd] = g2c1_ar[hd]
    l20 = np.zeros((64, ROW), dtype=np.float32)
    l20[:, 0:16] = s * g2c1_W
    l20[:, 16:18] = s * (g2c1_W @ al16)
    l20[:, 18:20] = s * (g2c1_W @ ar16)
    vl = (g2c2_W @ g2c2_al[0]).astype(np.float32)
    vr = (g2c2_W @ g2c2_ar[0]).astype(np.float32)

    maps = []
    for cc in range(NCORES):
        ft = np.zeros((64, c.nloc), dtype=np.float32)
        lo = cc * c.shard_real
        ft[:dfeat, :c.shard_real] = feat0[lo:lo + c.shard_real].T
        ft[dfeat, :] = 1.0
        maps.append({
            "featT": ft,
            "waug": waug,
            "l20": l20,
            "b1ext": np.tile(g2c1_b.reshape(1, 16), (P, 1)),
            "vlext": np.tile(vl.reshape(1, 16), (P, 1)),
            "vrext": np.tile(vr.reshape(1, 16), (P, 1)),
            "gidx": plan.gidx_cat[cc],
            "eridx": plan.eridx_cat[cc],
            "scidx": plan.scidx_cat[cc],
        })
    return maps


def host_tail(cfg, inputs, spartials):
    S = np.zeros(16, dtype=np.float64)
    for cc in range(NCORES):
        S += spartials[cc][:, 0].astype(np.float64)
    n_nodes = NCORES * cfg.shard_real
    W2 = np.asarray(inputs["g2c2_W"], dtype=np.float64)
    b2 = np.asarray(inputs["g2c2_b"], dtype=np.float64)
    mean = (S @ W2) / n_nodes + b2
    h = np.maximum(mean, 0.0)
    h = np.maximum(
        h @ np.asarray(inputs["fc1_w"], dtype=np.float64).T
        + np.asarray(inputs["fc1_b"], dtype=np.float64), 0.0)
    out = (h @ np.asarray(inputs["out_w"], dtype=np.float64).T
           + np.asarray(inputs["out_b"], dtype=np.float64))
    return out.astype(np.float32).reshape(1)


def kernel(**inputs):
    cfg = FULL
    src0 = np.asarray(inputs["src0"])
    dst0 = np.asarray(inputs["dst0"])
    plan, nc = _get(cfg, src0, dst0)
    in_maps = make_in_maps(cfg, plan, inputs)
    res = bass_utils.run_bass_kernel_spmd(nc, in_maps,
                                          core_ids=list(range(NCORES)))
    return host_tail(cfg, inputs, [res.results[cc]["spartial"]
                                   for cc in range(NCORES)])


# revision 19
# speedup vs baseline: 1.7526x; 1.1034x over previous
"""Trainium2 Bass kernel for nn_GAT_78151224918248 (gnn_message_passing).

Only the g0 branch of the reference is live (the g1 branch's output `ef` is
discarded), so the kernel computes
    nf0  = sqrt(2/64)*cos(feat0 @ W_rbf0 + b_rbf0)
    h0   = relu(gat_conv(nf0, g2c1_*))        # H=2, F=8
    out2 = gat_conv(h0, g2c2_*)               # H=1, F=64
    y    = MLP(relu(mean(out2, axis=0)))
and the final scalar is assembled on the host from per-core [16] partial sums
(g2c2_W is pulled out of the segment sums by linearity, so only 16-wide node
messages are aggregated on-device).

Distribution: nodes are sharded 25000/core across 8 NeuronCores (dst-major
edge sharding). Per layer, a 20-float node-record table is AllGathered; edges
are processed in 7 "rounds" by source-row range (to fit dma_gather's int16
indices), each round with a degree-sorted slot grid so all per-edge math is
plain broadcast/reduce vector work; per-round node partial sums are folded
into a DRAM accumulator with dma_scatter_add. Gather/scatter calls are one
per round (SWDGE streams descriptors through the ring with backpressure).
"""
import sys

for _p in ("/opt/trn_rl_repo", "/opt/pypackages"):
    if _p not in sys.path:
        sys.path.insert(0, _p)

import math
import numpy as np

import concourse.bass as bass
import concourse.bacc as bacc
import concourse.tile as tile
from concourse import mybir
from concourse import bass_utils

F32 = mybir.dt.float32
I16 = mybir.dt.int16
AF = mybir.ActivationFunctionType
ALU = mybir.AluOpType
AX = mybir.AxisListType

NCORES = 8
P = 128
TWO_PI = 2.0 * math.pi
PHASE_SHIFT = math.pi / 2.0
MAGIC = 12582912.0  # 1.5*2^23: fp32 add/sub rounds to nearest int

ROW = 20        # floats per table-row record
RSTRIDE = 64    # 256B stride of gatherable tables
ACC_W = 18      # floats scatter-added per node


class Cfg:
    def __init__(self, shard_real, jcount, nrounds):
        self.shard_real = shard_real
        self.jcount = jcount
        self.nloc = P * jcount
        assert shard_real <= self.nloc and shard_real >= (jcount - 1) * P
        self.nfull = NCORES * self.nloc
        self.nrounds = nrounds
        assert self.nfull % nrounds == 0
        self.rng_rows = self.nfull // nrounds
        assert self.rng_rows <= 32767


FULL = Cfg(shard_real=25000, jcount=196, nrounds=7)


def _wrap_idx16(vals):
    """[n] ints -> [128, ceil(n/16)] int16 SWDGE idx layout (idx i at
    [i%16, i//16], replicated to the 8 16-partition groups)."""
    n = len(vals)
    w = (n + 15) // 16
    pad = np.zeros(w * 16, dtype=np.int64)
    pad[:n] = vals
    a = np.zeros((P, w), dtype=np.int16)
    blk = pad.astype(np.int16).reshape(w, 16).T
    for g in range(8):
        a[g * 16:(g + 1) * 16, :] = blk
    return a


class Plan:
    """Host-side graph preprocessing shared by both layers."""

    def __init__(self, cfg: Cfg, src: np.ndarray, dst: np.ndarray):
        c = cfg
        self.cfg = c
        n_nodes = NCORES * c.shard_real
        src = src.astype(np.int64)
        dst = dst.astype(np.int64)
        assert src.min() >= 0 and src.max() < n_nodes
        assert dst.min() >= 0 and dst.max() < n_nodes

        core_of = dst // c.shard_real
        rows_of = (src // c.shard_real) * c.nloc + (src % c.shard_real)
        dloc = dst % c.shard_real
        rnd_of = rows_of // c.rng_rows

        # a pad table row inside every round's range (gather dummy target)
        pad_rows = np.concatenate(
            [cc * c.nloc + np.arange(c.shard_real, c.nloc) for cc in range(NCORES)])
        self.dummy = np.zeros(c.nrounds, dtype=np.int64)
        for r in range(c.nrounds):
            in_r = pad_rows[(pad_rows >= r * c.rng_rows)
                            & (pad_rows < (r + 1) * c.rng_rows)]
            assert len(in_r) > 0, f"no pad row available for round {r}"
            self.dummy[r] = in_r[0]

        # per (core, round) degree tables and node orders
        deg = np.zeros((NCORES, c.nrounds, c.nloc), dtype=np.int64)
        np.add.at(deg, (core_of, rnd_of, dloc), 1)
        orders = [[None] * c.nrounds for _ in range(NCORES)]
        for cc in range(NCORES):
            for r in range(c.nrounds):
                d = deg[cc, r]
                act = np.nonzero(d)[0]
                orders[cc][r] = act[np.argsort(-d[act], kind="stable")]

        # group templates shared across cores
        self.ng = np.zeros(c.nrounds, dtype=np.int64)
        self.widths = []
        for r in range(c.nrounds):
            ng_r = max((len(orders[cc][r]) + P - 1) // P for cc in range(NCORES))
            w_r = np.zeros(max(ng_r, 1), dtype=np.int64)[:ng_r]
            for cc in range(NCORES):
                o = orders[cc][r]
                if len(o) == 0:
                    continue
                ds = deg[cc, r][o]
                padded = np.zeros(ng_r * P, dtype=np.int64)
                padded[:len(ds)] = ds
                w_r = np.maximum(w_r, padded.reshape(ng_r, P).max(axis=1))
            self.ng[r] = ng_r
            self.widths.append(w_r)

        # batches: (round, g0, ngb, w, col0) = maximal equal-width group runs
        self.batches = []
        self.cols = np.zeros(c.nrounds, dtype=np.int64)
        for r in range(c.nrounds):
            w_r = self.widths[r]
            col = 0
            g = 0
            while g < len(w_r):
                w = int(w_r[g])
                g2 = g
                while g2 < len(w_r) and int(w_r[g2]) == w:
                    g2 += 1
                self.batches.append((r, g, g2 - g, w, col))
                col += (g2 - g) * w
                g = g2
            self.cols[r] = col

        # per-core index arrays
        trash = c.shard_real  # local pad row for scatter padding
        self.gidx_cat, self.eridx_cat, self.scidx_cat = [], [], []
        for cc in range(NCORES):
            g_parts, er_parts, sc_parts = [], [], []
            for r in range(c.nrounds):
                ng_r = int(self.ng[r])
                w_r = self.widths[r]
                cols_r = int(self.cols[r])
                o = orders[cc][r]
                nact = len(o)
                gvals = np.full(cols_r * P, self.dummy[r], dtype=np.int64)
                ervals = np.zeros(ng_r * P, dtype=np.int64)
                scvals = np.full(ng_r * P, trash, dtype=np.int64)
                if nact:
                    ervals[:nact] = o
                    scvals[:nact] = o
                    # edges of (cc, r) sorted by node position
                    m = (core_of == cc) & (rnd_of == r)
                    ed, er_rows = dloc[m], rows_of[m]
                    pos_of = np.full(c.nloc, -1, dtype=np.int64)
                    pos_of[o] = np.arange(nact)
                    pe = pos_of[ed]
                    si = np.argsort(pe, kind="stable")
                    pe, er_rows = pe[si], er_rows[si]
                    # k = intra-node running index
                    firsts = np.searchsorted(pe, np.arange(nact))
                    k = np.arange(len(pe)) - firsts[pe]
                    col0_of_g = np.concatenate([[0], np.cumsum(w_r)])[:-1]
                    gg, pp = pe // P, pe % P
                    slot = (col0_of_g[gg] + k) * P + pp
                    gvals[slot] = er_rows
                g_parts.append(_wrap_idx16(gvals - r * c.rng_rows))
                er_parts.append(_wrap_idx16(ervals))
                sc_parts.append(_wrap_idx16(scvals))
            self.gidx_cat.append(np.concatenate(g_parts, axis=1))
            self.eridx_cat.append(np.concatenate(er_parts, axis=1))
            self.scidx_cat.append(np.concatenate(sc_parts, axis=1))
        self.gidx_off = np.concatenate(
            [[0], np.cumsum([_wrap_idx16(np.zeros(int(self.cols[r]) * P)).shape[1]
                             for r in range(c.nrounds)])])
        self.eridx_off = np.concatenate(
            [[0], np.cumsum([int(self.ng[r]) * 8 for r in range(c.nrounds)])])


def patch_dma_gather():
    import inspect
    import textwrap
    b = bass
    if getattr(b.BassGpSimd.dma_gather, "_flex_patched", False):
        return
    src = textwrap.dedent(inspect.getsource(b.BassGpSimd.dma_gather))
    bad = ("assert (\n        elem_size_bytes > 0 and elem_size_bytes % 256 == 0\n"
           "    )  # transpose restriction")
    assert bad in src, "dma_gather source changed; fix patch"
    src = src.replace(bad, "assert elem_size_bytes > 0")
    ns = dict(vars(b))
    exec(src, ns)
    ns["dma_gather"]._flex_patched = True
    b.BassGpSimd.dma_gather = ns["dma_gather"]


def _apx(base_ap, extra_off, dims):
    """New AP on the same tensor: keep partition dim, replace free dims."""
    return bass.AP(tensor=base_ap.tensor, offset=base_ap.offset + extra_off,
                   ap=[list(base_ap.ap[0])] + [list(d) for d in dims])


def build_program(cfg: Cfg, plan: Plan, nreps: int = 1, probe: str = ""):
    patch_dma_gather()
    c = cfg
    J, NL, NF = c.jcount, c.nloc, c.nfull
    ng_max = int(max(plan.ng))
    cols_max = int(max(plan.cols))
    nqueues = 1 if "q1" in probe else (2 if "q2" in probe else 4)
    nc = bacc.Bacc("TRN2", target_bir_lowering=False, debug=False,
                   num_devices=NCORES, dynamic_dma_scratch_size=65536,
                   num_swdge_queues=nqueues)
    nc._kernel_nqueues = nqueues

    featT = nc.dram_tensor("featT", [64, NL], F32, kind="ExternalInput")
    waug = nc.dram_tensor("waug", [64, 64], F32, kind="ExternalInput")
    l20 = nc.dram_tensor("l20", [64, ROW], F32, kind="ExternalInput")
    b1ext = nc.dram_tensor("b1ext", [P, 16], F32, kind="ExternalInput")
    vlext = nc.dram_tensor("vlext", [P, 16], F32, kind="ExternalInput")
    vrext = nc.dram_tensor("vrext", [P, 16], F32, kind="ExternalInput")
    gidx_h = nc.dram_tensor("gidx", [P, int(plan.gidx_off[-1])], I16,
                            kind="ExternalInput")
    eridx_h = nc.dram_tensor("eridx", [P, int(plan.eridx_off[-1])], I16,
                             kind="ExternalInput")
    scidx_h = nc.dram_tensor("scidx", [P, int(plan.eridx_off[-1])], I16,
                             kind="ExternalInput")
    spartial = nc.dram_tensor("spartial", [16, 1], F32, kind="ExternalOutput")

    tab1_loc = nc.dram_tensor("tab1_loc", [NL, RSTRIDE], F32)
    tab2_loc = nc.dram_tensor("tab2_loc", [NL, RSTRIDE], F32)
    gtab1 = nc.dram_tensor("gtab1", [NF, RSTRIDE], F32, addr_space="Shared")
    gtab2 = nc.dram_tensor("gtab2", [NF, RSTRIDE], F32, addr_space="Shared")
    acc1 = nc.dram_tensor("acc1", [NL, RSTRIDE], F32)
    acc2 = nc.dram_tensor("acc2", [NL, RSTRIDE], F32)

    pad_p0 = c.shard_real - (J - 1) * P  # pads are (p >= pad_p0, j == J-1)
    npad = NL - c.shard_real
    assert 0 < npad <= P

    with tile.TileContext(nc) as tc:
        with tc.tile_pool(name="persist", bufs=1) as pers:
            waug_sb = pers.tile([64, 64], F32)
            nc.sync.dma_start(out=waug_sb[:, :], in_=waug.ap())
            l20_sb = pers.tile([P, ROW], F32)
            nc.sync.dma_start(out=l20_sb[0:64, :], in_=l20.ap())
            nc.sync.dma_start(out=l20_sb[64:128, :], in_=l20.ap())
            b1_sb = pers.tile([P, 16], F32)
            nc.sync.dma_start(out=b1_sb[:, :], in_=b1ext.ap())
            vl_sb = pers.tile([P, 16], F32)
            nc.sync.dma_start(out=vl_sb[:, :], in_=vlext.ap())
            vr_sb = pers.tile([P, 16], F32)
            nc.sync.dma_start(out=vr_sb[:, :], in_=vrext.ap())
            zero_sb = pers.tile([P, 2048], F32)
            nc.vector.memset(zero_sb[:, :], 0.0)
            # pad-row record for layer1/layer2 tables: zeros except el=-1e30
            pad1_sb = pers.tile([P, ROW], F32)
            nc.vector.memset(pad1_sb[:, :], 0.0)
            nc.vector.memset(pad1_sb[:, 16:18], -1.0e30)
            pad2_sb = pers.tile([P, ROW], F32)
            nc.vector.memset(pad2_sb[:, :], 0.0)
            nc.vector.memset(pad2_sb[:, 16:17], -1.0e30)
            hub1 = pers.tile([P, J, ROW], F32)
            hub2 = pers.tile([P, J, ROW], F32)

            T = dict(featT=featT, waug_sb=waug_sb, l20_sb=l20_sb,
                     b1_sb=b1_sb, vl_sb=vl_sb, vr_sb=vr_sb,
                     zero_sb=zero_sb, pad1_sb=pad1_sb,
                     pad2_sb=pad2_sb, hub1=hub1, hub2=hub2,
                     gidx_h=gidx_h, eridx_h=eridx_h, scidx_h=scidx_h,
                     tab1_loc=tab1_loc, tab2_loc=tab2_loc,
                     gtab1=gtab1, gtab2=gtab2, acc1=acc1, acc2=acc2,
                     spartial=spartial)
            T["probe"] = probe
            for rep in range(nreps):
                body_once(nc, tc, c, plan, rep, T, ng_max, cols_max)

    nc.compile()
    return nc


CALL_COLS = 28  # 3584 indices per SWDGE call (ring is 4096 descriptors)
STAGE_J = 28    # J-chunk per staged 64-wide table write


def store_table(nc, tc, c, rep, layer, hub, tab_loc, pad_sb):
    """hub [P, J, ROW] -> tab_loc [NL, RSTRIDE] (contiguous 256B-row writes;
    pad columns carry garbage, never read). Row index of node (p, j) is
    j*128 + p, matching the gather index construction."""
    J = c.jcount
    npad = c.nloc - c.shard_real
    with tc.tile_pool(name=f"st{layer}_{rep}", bufs=2) as stp:
        for j0 in range(0, J, STAGE_J):
            jn = min(STAGE_J, J - j0)
            st = stp.tile([P, STAGE_J, RSTRIDE], F32, tag="st")
            nc.vector.tensor_copy(out=st[:, 0:jn, 0:ROW],
                                  in_=hub[:, j0:j0 + jn, :])
            nc.sync.dma_start(
                out=bass.AP(tensor=tab_loc, offset=j0 * P * RSTRIDE,
                            ap=[[RSTRIDE, P], [RSTRIDE * P, jn], [1, RSTRIDE]]),
                in_=st[:, 0:jn, :])
    nc.sync.dma_start(
        out=bass.AP(tensor=tab_loc, offset=c.shard_real * RSTRIDE,
                    ap=[[RSTRIDE, npad], [1, ROW]]),
        in_=pad_sb[0:npad, :])


def body_once(nc, tc, c, plan, rep, T, ng_max, cols_max):
    call_cols = 7 if "smallcall" in T["probe"] else CALL_COLS
    J, NL, NF = c.jcount, c.nloc, c.nfull
    npad = NL - c.shard_real

    nq = getattr(nc, "_kernel_nqueues", 1)

    qchains = T.setdefault("swdge_qchains", [[] for _ in range(nq)])
    qrr = T.setdefault("swdge_qrr", [0])

    def next_q():
        q = qrr[0] % nq
        qrr[0] += 1
        return q

    def chain(inst, q):
        ch = qchains[q]
        if len(ch) >= 2:
            bass._add_dep_helper(inst.ins, ch[-2].ins, sync=True,
                                 reason="swdge ring throttle")
        ch.append(inst)
        return inst
    featT, zero_sb = T["featT"], T["zero_sb"]
    hub1, hub2 = T["hub1"], T["hub2"]
    tab1_loc, tab2_loc = T["tab1_loc"], T["tab2_loc"]
    gtab1, gtab2 = T["gtab1"], T["gtab2"]
    acc1, acc2 = T["acc1"], T["acc2"]

    # ---------- phase 0: RBF + layer-1 node records ----------
    half = NL // 2
    assert half % P == 0
    blk = 1792 if half % 1792 == 0 else P
    nblk = half // blk
    jt_per_blk = blk // P
    with tc.tile_pool(name=f"ph0_{rep}", bufs=3) as ph0, \
         tc.tile_pool(name=f"nf0p_{rep}", bufs=2) as nf0p, \
         tc.tile_pool(name=f"ph0ps_{rep}", bufs=2, space="PSUM") as ph0ps, \
         tc.tile_pool(name=f"zhps_{rep}", bufs=4, space="PSUM") as zhps:
        for b in range(nblk):
            c0 = b * blk
            nf0_t = nf0p.tile([P, blk], F32, tag="nf0")
            for ch0 in range(0, blk, 512):
                cw = min(512, blk - ch0)
                ft = ph0.tile([64, 512], F32, tag="ft")
                nc.sync.dma_start(out=ft[:, :cw],
                                  in_=featT.ap()[:, c0 + ch0:c0 + ch0 + cw])
                ft2 = ph0.tile([64, 512], F32, tag="ft2")
                nc.sync.dma_start(
                    out=ft2[:, :cw],
                    in_=featT.ap()[:, half + c0 + ch0:half + c0 + ch0 + cw])
                ps = ph0ps.tile([P, 512], F32, space="PSUM", tag="ps")
                nc.tensor.matmul(ps[0:64, :cw], T["waug_sb"][:, :], ft[:, :cw],
                                 start=True, stop=True)
                nc.tensor.matmul(ps[64:128, :cw], T["waug_sb"][:, :],
                                 ft2[:, :cw], start=True, stop=True)
                wt = ph0.tile([P, 512], F32, tag="wt")
                kt = ph0.tile([P, 512], F32, tag="kt")
                # k = round(z / 2pi) via the fp32 magic constant
                nc.vector.tensor_scalar(out=kt[:, :cw], in0=ps[:, :cw],
                                        scalar1=1.0 / TWO_PI, scalar2=MAGIC,
                                        op0=ALU.mult, op1=ALU.add)
                nc.vector.tensor_scalar_add(out=kt[:, :cw], in0=kt[:, :cw],
                                            scalar1=-MAGIC)
                # w = z - k*2pi, clamped into the Sin LUT domain
                nc.vector.scalar_tensor_tensor(
                    out=wt[:, :cw], in0=kt[:, :cw], scalar=-TWO_PI,
                    in1=ps[:, :cw], op0=ALU.mult, op1=ALU.add)
                nc.vector.tensor_scalar(out=wt[:, :cw], in0=wt[:, :cw],
                                        scalar1=math.pi * 0.9999999,
                                        scalar2=-math.pi * 0.9999999,
                                        op0=ALU.min, op1=ALU.max)
                nc.scalar.activation(nf0_t[:, ch0:ch0 + cw], wt[:, :cw],
                                     AF.Sin)
            for hs in range(2):
                zb = zhps.tile([P, jt_per_blk, ROW], F32, space="PSUM",
                               tag="zb")
                for jj in range(jt_per_blk):
                    nc.tensor.matmul(
                        zb[:, jj, :],
                        nf0_t[hs * 64:(hs + 1) * 64, jj * P:(jj + 1) * P],
                        T["l20_sb"][hs * 64:(hs + 1) * 64, :],
                        start=True, stop=True)
                jbase = (hs * half + c0) // P
                nc.scalar.activation(hub1[:, jbase:jbase + jt_per_blk, :],
                                     zb[:, :, :], AF.Identity)
    store_table(nc, tc, c, rep, 1, hub1, tab1_loc, T["pad1_sb"])

    if "noag" in T["probe"]:
        for cc8 in range(NCORES):
            nc.sync.dma_start(out=gtab1.ap()[cc8 * NL:(cc8 + 1) * NL, :],
                              in_=tab1_loc.ap())
    else:
        nc.gpsimd.collective_compute(
            "AllGather", ALU.bypass, replica_groups=[list(range(NCORES))],
            ins=[tab1_loc.ap()], outs=[gtab1.ap()])

    # zero accumulators (runs alongside the collective)
    for a in (acc1, acc2):
        tot = NL * RSTRIDE
        step = P * 2048
        off = 0
        while off < tot:
            sz = min(step, tot - off)
            assert sz % P == 0
            q = sz // P
            v = bass.AP(tensor=a, offset=off, ap=[[q, P], [1, q]])
            nc.sync.dma_start(out=v, in_=zero_sb[:, :q])
            off += sz

    # ---------- edge stage ----------
    def edge_layer(layer):
        gtab = gtab1 if layer == 1 else gtab2
        tab_loc = tab1_loc if layer == 1 else tab2_loc
        acc = acc1 if layer == 1 else acc2
        rlen = 18 if layer == 1 else 17
        nh = 2 if layer == 1 else 1
        er_view = bass.AP(tensor=tab_loc, offset=18 if layer == 1 else 17,
                          ap=[[RSTRIDE, NL], [1, 2]])
        acc_view = bass.AP(tensor=acc, offset=0,
                           ap=[[RSTRIDE, NL], [1, ACC_W]])
        with tc.tile_pool(name=f"l{layer}idx_{rep}", bufs=2) as idxp, \
             tc.tile_pool(name=f"l{layer}er_{rep}", bufs=1) as erp, \
             tc.tile_pool(name=f"l{layer}g_{rep}", bufs=2) as gp, \
             tc.tile_pool(name=f"l{layer}w_{rep}", bufs=2) as wp, \
             tc.tile_pool(name=f"l{layer}acc_{rep}", bufs=2) as accp:
            # hoist ALL er gathers: they read only the LOCAL table, so their
            # Q7 descriptor generation overlaps the AllGather window that the
            # G gathers must wait for.
            erts = {}
            for r in range(c.nrounds):
                ng_r = int(plan.ng[r])
                if ng_r == 0:
                    continue
                ei0 = int(plan.eridx_off[r])
                eridx_t = erp.tile([P, ng_max * 8], I16, tag=f"eridx{r}")
                nc.sync.dma_start(out=eridx_t[:, :ng_r * 8],
                                  in_=T["eridx_h"].ap()[:, ei0:ei0 + ng_r * 8])
                ert = erp.tile([P, ng_max, 2], F32, tag=f"ert{r}")
                erts[r] = ert
                for q0 in range(0, ng_r, call_cols):
                    qn = min(call_cols, ng_r - q0)
                    q = next_q()
                    chain(nc.gpsimd.dma_gather(
                        ert[:, q0:q0 + qn, :], er_view,
                        eridx_t[:, q0 * 8:(q0 + qn) * 8],
                        qn * P, qn * P, 2, elem_step=RSTRIDE,
                        single_packet=False, queue_num=q), q)
            for r in range(c.nrounds):
                ng_r = int(plan.ng[r])
                cols_r = int(plan.cols[r])
                if ng_r == 0:
                    continue
                gt_view = bass.AP(
                    tensor=gtab, offset=r * c.rng_rows * RSTRIDE,
                    ap=[[RSTRIDE, c.rng_rows], [1, rlen]])
                gi0 = int(plan.gidx_off[r])
                ei0 = int(plan.eridx_off[r])
                gw = (cols_r * P + 15) // 16
                gidx_t = idxp.tile([P, (cols_max * P + 15) // 16], I16,
                                   tag="gidx")
                nc.sync.dma_start(out=gidx_t[:, :gw],
                                  in_=T["gidx_h"].ap()[:, gi0:gi0 + gw])
                scidx_t = idxp.tile([P, ng_max * 8], I16, tag="scidx")
                nc.sync.dma_start(out=scidx_t[:, :ng_r * 8],
                                  in_=T["scidx_h"].ap()[:, ei0:ei0 + ng_r * 8])
                ert = erts[r]

                G = gp.tile([P, cols_max, rlen], F32, tag="G")
                for q0 in range(0, cols_r, call_cols):
                    qn = min(call_cols, cols_r - q0)
                    q = next_q()
                    chain(nc.gpsimd.dma_gather(
                        G[:, q0:q0 + qn, :], gt_view,
                        gidx_t[:, q0 * 8:(q0 + qn) * 8],
                        qn * P, qn * P, rlen, elem_step=RSTRIDE,
                        single_packet=False, queue_num=q), q)

                acc_t = accp.tile([P, ng_max, ACC_W], F32, tag="acc")

                for (br, g0, ngb, w, col0) in plan.batches:
                    if br != r:
                        continue
                    cols_b = ngb * w
                    Gb = G[:, col0:col0 + cols_b, :]
                    tt = wp.tile([P, cols_max * 2], F32, tag="tt")
                    at = wp.tile([P, cols_max * 2], F32, tag="at")
                    ert_b = ert[:, 0:ng_r, :]
                    if layer == 1:
                        el_ap = _apx(Gb, 16, [[rlen * w, ngb], [rlen, w],
                                              [1, 2]])
                        er_ap = _apx(ert_b, g0 * 2, [[2, ngb], [0, w],
                                                     [1, 2]])
                        t_ap = _apx(tt[:, :], 0, [[2 * w, ngb], [1, w],
                                                  [w, 2]])
                        nact = cols_b * 2
                    else:
                        el_ap = _apx(Gb, 16, [[rlen, cols_b]])
                        er_ap = _apx(ert_b, g0 * 2, [[2, ngb], [0, w]])
                        t_ap = tt[:, 0:cols_b]
                        nact = cols_b
                    nc.vector.tensor_tensor(out=t_ap, in0=el_ap,
                                            in1=er_ap, op=ALU.add)
                    # leaky_relu(t, 0.2) = max(0.2*t, t)
                    nc.vector.scalar_tensor_tensor(
                        out=tt[:, 0:nact], in0=tt[:, 0:nact], scalar=0.2,
                        in1=tt[:, 0:nact], op0=ALU.mult, op1=ALU.max)
                    nc.scalar.activation(at[:, 0:nact], tt[:, 0:nact],
                                         AF.Exp)
                    V2 = wp.tile([P, cols_max, 16], F32, tag="V2")
                    for hd in range(nh):
                        fw = 16 // nh
                        h_ap = _apx(Gb, hd * fw, [[rlen * w, ngb],
                                                  [rlen, w], [1, fw]])
                        if layer == 1:
                            a_ap = _apx(at[:, :], hd * w,
                                        [[2 * w, ngb], [1, w], [0, fw]])
                            v_ap = _apx(V2[:, :, :], hd * fw * w,
                                        [[16 * w, ngb], [1, w], [w, fw]])
                        else:
                            a_ap = _apx(at[:, :], 0,
                                        [[w, ngb], [1, w], [0, fw]])
                            v_ap = _apx(V2[:, :, :], 0,
                                        [[16 * w, ngb], [1, w], [w, fw]])
                        nc.vector.tensor_tensor(out=v_ap, in0=h_ap,
                                                in1=a_ap, op=ALU.mult)
                    vred = _apx(V2[:, :, :], 0, [[16 * w, ngb], [w, 16],
                                                 [1, w]])
                    m_ap = _apx(acc_t[:, :, :], g0 * ACC_W,
                                [[ACC_W, ngb], [1, 16]])
                    nc.vector.tensor_reduce(out=m_ap, in_=vred, axis=AX.X,
                                            op=ALU.add)
                    if layer == 1:
                        den_in = _apx(at[:, :], 0, [[2 * w, ngb], [w, 2],
                                                    [1, w]])
                        den_out = _apx(acc_t[:, :, :], g0 * ACC_W + 16,
                                       [[ACC_W, ngb], [1, 2]])
                    else:
                        den_in = _apx(at[:, :], 0, [[w, ngb], [1, w]])
                        den_out = _apx(acc_t[:, :, :], g0 * ACC_W + 16,
                                       [[ACC_W, ngb]])
                    nc.vector.tensor_reduce(out=den_out, in_=den_in,
                                            axis=AX.X, op=ALU.add)
                    if layer == 2:
                        # unused den slot: keep deterministic zeros
                        pass

                for q0 in range(0, ng_r, call_cols):
                    qn = min(call_cols, ng_r - q0)
                    q = next_q()
                    chain(nc.gpsimd.dma_scatter_add(
                        acc_view, acc_t[:, q0:q0 + qn, :],
                        scidx_t[:, q0 * 8:(q0 + qn) * 8],
                        qn * P, qn * P, ACC_W, elem_step=RSTRIDE,
                        single_packet=False, queue_num=q), q)

    if "noedge" not in T["probe"]:
        edge_layer(1)

    # ---------- layer-1 finalize: h0 / el2 / er2 -> table2 ----------
    with tc.tile_pool(name=f"fin1_{rep}", bufs=1) as fin:
        accl = fin.tile([P, J, ACC_W], F32)
        nc.sync.dma_start(
            out=bass.AP(tensor=acc1, offset=c.shard_real * RSTRIDE,
                        ap=[[RSTRIDE, npad], [1, RSTRIDE]]),
            in_=zero_sb[0:npad, 0:RSTRIDE])
        nc.sync.dma_start(
            out=accl[:, :, :],
            in_=bass.AP(tensor=acc1, offset=0,
                        ap=[[RSTRIDE, P], [RSTRIDE * P, J], [1, ACC_W]]))
        dmax = fin.tile([P, J, 2], F32)
        nc.vector.tensor_scalar_max(out=dmax[:, :, :], in0=accl[:, :, 16:18],
                                    scalar1=1e-9)
        rec = fin.tile([P, J, 2], F32)
        nc.vector.reciprocal(out=rec[:, :, :], in_=dmax[:, :, :])
        h0p = fin.tile([P, J, 16], F32)
        rec_b = _apx(rec[:, :, :], 0, [[2, J], [1, 2], [0, 8]])
        nc.vector.tensor_tensor(out=h0p[:, :, :], in0=accl[:, :, 0:16],
                                in1=rec_b, op=ALU.mult)
        b1_b = _apx(T["b1_sb"][:, :], 0, [[0, J], [1, 16]])
        nc.vector.tensor_tensor(out=h0p[:, :, :], in0=h0p[:, :, :], in1=b1_b,
                                op=ALU.add)
        nc.vector.tensor_scalar_max(out=hub2[:, :, 0:16], in0=h0p[:, :, :],
                                    scalar1=0.0)
        tmp = fin.tile([P, J, 16], F32)
        vl_b = _apx(T["vl_sb"][:, :], 0, [[0, J], [1, 16]])
        nc.vector.tensor_tensor(out=tmp[:, :, :], in0=hub2[:, :, 0:16],
                                in1=vl_b, op=ALU.mult)
        nc.vector.tensor_reduce(out=hub2[:, :, 16], in_=tmp[:, :, :],
                                axis=AX.X, op=ALU.add)
        vr_b = _apx(T["vr_sb"][:, :], 0, [[0, J], [1, 16]])
        nc.vector.tensor_tensor(out=tmp[:, :, :], in0=hub2[:, :, 0:16],
                                in1=vr_b, op=ALU.mult)
        nc.vector.tensor_reduce(out=hub2[:, :, 17], in_=tmp[:, :, :],
                                axis=AX.X, op=ALU.add)
        nc.vector.memset(hub2[:, :, 18:20], 0.0)
    store_table(nc, tc, c, rep, 2, hub2, tab2_loc, T["pad2_sb"])

    if "noag" in T["probe"]:
        for cc8 in range(NCORES):
            nc.sync.dma_start(out=gtab2.ap()[cc8 * NL:(cc8 + 1) * NL, :],
                              in_=tab2_loc.ap())
    else:
        nc.gpsimd.collective_compute(
            "AllGather", ALU.bypass, replica_groups=[list(range(NCORES))],
            ins=[tab2_loc.ap()], outs=[gtab2.ap()])

    if "noedge" not in T["probe"]:
        edge_layer(2)

    # ---------- layer-2 finalize -> spartial ----------
    with tc.tile_pool(name=f"fin2_{rep}", bufs=1) as fin, \
         tc.tile_pool(name=f"fps_{rep}", bufs=1, space="PSUM") as fps:
        acc2l = fin.tile([P, J, 17], F32)
        nc.sync.dma_start(
            out=bass.AP(tensor=acc2, offset=c.shard_real * RSTRIDE,
                        ap=[[RSTRIDE, npad], [1, RSTRIDE]]),
            in_=zero_sb[0:npad, 0:RSTRIDE])
        nc.sync.dma_start(
            out=acc2l[:, :, :],
            in_=bass.AP(tensor=acc2, offset=0,
                        ap=[[RSTRIDE, P], [RSTRIDE * P, J], [1, 17]]))
        d2 = fin.tile([P, J], F32)
        nc.vector.tensor_scalar_max(out=d2[:, :], in0=acc2l[:, :, 16],
                                    scalar1=1e-9)
        r2 = fin.tile([P, J], F32)
        nc.vector.reciprocal(out=r2[:, :], in_=d2[:, :])
        rt = fin.tile([P, 16, J], F32)
        r2_b = _apx(r2[:, :], 0, [[1, J], [0, 16]])
        rt_ap = _apx(rt[:, :, :], 0, [[1, J], [J, 16]])
        nc.vector.tensor_tensor(out=rt_ap, in0=acc2l[:, :, 0:16], in1=r2_b,
                                op=ALU.mult)
        S_acc = fin.tile([P, 16], F32)
        nc.vector.tensor_reduce(out=S_acc[:, :], in_=rt[:, :, :], axis=AX.X,
                                op=ALU.add)
        ones = fin.tile([P, 1], F32)
        nc.vector.memset(ones[:, :], 1.0)
        sp = fps.tile([16, 1], F32, space="PSUM")
        nc.tensor.matmul(sp[:, :], S_acc[:, :], ones[:, :], start=True,
                         stop=True)
        sout = fin.tile([16, 1], F32)
        nc.vector.tensor_copy(out=sout[:, :], in_=sp[:, :])
        nc.sync.dma_start(out=T["spartial"].ap(), in_=sout[:, :])


# ---------------- host orchestration ----------------

_CACHE = {}


def _get(cfg, src0, dst0, nreps=1, probe=""):
    key = (cfg.shard_real, cfg.jcount, cfg.nrounds, nreps, probe,
           hash(src0.tobytes()), hash(dst0.tobytes()))
    if key not in _CACHE:
        plan = Plan(cfg, src0, dst0)
        nc = build_program(cfg, plan, nreps=nreps, probe=probe)
        _CACHE[key] = (plan, nc)
    return _CACHE[key]


def make_in_maps(cfg, plan, inputs):
    c = cfg
    s = math.sqrt(2.0 / 64.0)
    feat0 = np.asarray(inputs["feat0"], dtype=np.float32)
    W_rbf0 = np.asarray(inputs["W_rbf0"], dtype=np.float32)
    b_rbf0 = np.asarray(inputs["b_rbf0"], dtype=np.float32)
    g2c1_W = np.asarray(inputs["g2c1_W"], dtype=np.float32)
    g2c1_al = np.asarray(inputs["g2c1_al"], dtype=np.float32)
    g2c1_ar = np.asarray(inputs["g2c1_ar"], dtype=np.float32)
    g2c1_b = np.asarray(inputs["g2c1_b"], dtype=np.float32)
    g2c2_W = np.asarray(inputs["g2c2_W"], dtype=np.float32)
    g2c2_al = np.asarray(inputs["g2c2_al"], dtype=np.float32)
    g2c2_ar = np.asarray(inputs["g2c2_ar"], dtype=np.float32)

    dfeat = feat0.shape[1]
    waug = np.zeros((64, 64), dtype=np.float32)
    waug[:dfeat, :] = W_rbf0
    waug[dfeat, :] = b_rbf0 + PHASE_SHIFT
    al16 = np.zeros((16, 2), dtype=np.float32)
    ar16 = np.zeros((16, 2), dtype=np.float32)
    for hd in range(2):
        al16[hd * 8:(hd + 1) * 8, hd] = g2c1_al[hd]
        ar16[hd * 8:(hd + 1) * 8, hd] = g2c1_ar[hd]
    l20 = np.zeros((64, ROW), dtype=np.float32)
    l20[:, 0:16] = s * g2c1_W
    l20[:, 16:18] = s * (g2c1_W @ al16)
    l20[:, 18:20] = s * (g2c1_W @ ar16)
    vl = (g2c2_W @ g2c2_al[0]).astype(np.float32)
    vr = (g2c2_W @ g2c2_ar[0]).astype(np.float32)

    maps = []
    for cc in range(NCORES):
        ft = np.zeros((64, c.nloc), dtype=np.float32)
        lo = cc * c.shard_real
        ft[:dfeat, :c.shard_real] = feat0[lo:lo + c.shard_real].T
        ft[dfeat, :] = 1.0
        maps.append({
            "featT": ft,
            "waug": waug,
            "l20": l20,
            "b1ext": np.tile(g2c1_b.reshape(1, 16), (P, 1)),
            "vlext": np.tile(vl.reshape(1, 16), (P, 1)),
            "vrext": np.tile(vr.reshape(1, 16), (P, 1)),
            "gidx": plan.gidx_cat[cc],
            "eridx": plan.eridx_cat[cc],
            "scidx": plan.scidx_cat[cc],
        })
    return maps


def host_tail(cfg, inputs, spartials):
    S = np.zeros(16, dtype=np.float64)
    for cc in range(NCORES):
        S += spartials[cc][:, 0].astype(np.float64)
    n_nodes = NCORES * cfg.shard_real
    W2 = np.asarray(inputs["g2c2_W"], dtype=np.float64)
    b2 = np.asarray(inputs["g2c2_b"], dtype=np.float64)
    mean = (S @ W2) / n_nodes + b2
    h = np.maximum(mean, 0.0)
    h = np.maximum(
        h @ np.asarray(inputs["fc1_w"], dtype=np.float64).T
        + np.asarray(inputs["fc1_b"], dtype=np.float64), 0.0)
    out = (h @ np.asarray(inputs["out_w"], dtype=np.float64).T
           + np.asarray(inputs["out_b"], dtype=np.float64))
    return out.astype(np.float32).reshape(1)


def kernel(**inputs):
    cfg = FULL
    src0 = np.asarray(inputs["src0"])
    dst0 = np.asarray(inputs["dst0"])
    plan, nc = _get(cfg, src0, dst0)
    in_maps = make_in_maps(cfg, plan, inputs)
    res = bass_utils.run_bass_kernel_spmd(nc, in_maps,
                                          core_ids=list(range(NCORES)))
    return host_tail(cfg, inputs, [res.results[cc]["spartial"]
                                   for cc in range(NCORES)])
